# revision 36
# baseline (speedup 1.0000x reference)
"""Bass/Trainium2 kernel for nn_BiLSTM_Tok_83837761618147.

Strategy (8 NeuronCores, SPMD, full inputs in / full output out):
  - Token dim sharded 8 ways (16384 tokens/core, with halos).
  - BiLSTM parallelized via chunked recurrence with burn-in: each core runs
    128 lanes x (128+64) steps forward and 128 lanes x (129+64) steps
    backward (state forgets exponentially; 64 warmup steps reach fp32
    accuracy; the true h0/c0-seeded lanes cover the sequence ends exactly).
  - Gate pre-activations computed by PE matmuls directly into PSUM
    (bias via a K=4 indicator matmul); w_hh @ h accumulated on top.
  - Attention (tanh/logits/exp) + ragged segment softmax-sum done on
    device via an e-weighted one-hot (token x segment-window) matmul.
  - Host combines per-core partial [segment, 257] sums, normalizes, and
    applies the tiny tag projection.
"""

import numpy as np
import ml_dtypes

BF16 = ml_dtypes.bfloat16

T = 131072
D = 256
H = 128
HID = 256
TAGS = 10
S = 1024
NCORE = 8
PC = T // NCORE          # 16384 tokens per core
B = 64                   # burn-in steps
LF = 128                 # forward lane length (tokens per lane)
LB = 129                 # backward lane length
NL = 128                 # lanes per direction
NSF = B + LF             # 192 forward steps
NSB = B + LB             # 193 backward steps
SH = 16640               # x shard rows [tc0-64, tc0-64+SH)
SWIN = 192               # segment window width per core (max observed 142)
NQ = PC                  # main attention window positions
NTILE = NQ // 128        # 128 main token tiles
HBW = LB * NL - LB + LB + B  # hbT width: 16512
HBT_W = 16512
ATT_W = NQ + 128         # att buffer width (main + extra tile)

_BUILT = {}
LAST_RESULT = None


def _build():
    if "nc" in _BUILT:
        return _BUILT["nc"]
    import contextlib
    from concourse import bacc, mybir
    from concourse.tile import TileContext

    F32 = mybir.dt.float32
    BF = mybir.dt.bfloat16
    AF = mybir.ActivationFunctionType
    ALU = mybir.AluOpType

    nc = bacc.Bacc()

    def din(name, shape, dt):
        return nc.declare_dram_parameter(name, list(shape), dt, isOutput=False)

    xsf_in = din("xSf", [256, 128 * NSF], BF)
    xsb_in = din("xSb", [256, 128 * NSB], BF)
    wih_f_in = din("wih_f", [256, 512], BF)
    wih_b_in = din("wih_b", [256, 512], BF)
    whh_f_in = din("whh_f", [128, 512], BF)
    whh_b_in = din("whh_b", [128, 512], BF)
    b48_in = din("b48", [128, 128], BF)
    i48_in = din("i48", [128, 1024], BF)
    h0f_in = din("h0f", [128, 128], BF)
    c0f_in = din("c0f", [128, 128], BF)
    h0b_in = din("h0b", [128, 128], BF)
    c0b_in = din("c0b", [128, 128], BF)
    wom_in = din("wom", [256, 256], BF)
    uo_in = din("uo", [256, 1], BF)
    iota_in = din("iota", [128, SWIN], BF)
    identb_in = din("identb", [128, 128], BF)
    identf_in = din("identf", [128, 128], F32)
    seg_in = din("seg", [128, 129], F32)
    ctx_out = nc.declare_dram_parameter("ctx", [SWIN, 257], F32, isOutput=True)
    att_dram = nc.dram_tensor("att_stage", [1, ATT_W], F32)

    with TileContext(nc) as tc, contextlib.ExitStack() as ctx:
        pp = ctx.enter_context(tc.tile_pool(name="persist", bufs=1))

        # chunked step-major x staging (CH steps per chunk, double buffered)
        CH = 16
        xf = [[pp.tile([128, 128 * CH], BF, tag=f"xf{b}{kh}", name=f"xf{b}{kh}")
               for kh in range(2)] for b in range(2)]
        xb = [[pp.tile([128, 128 * CH], BF, tag=f"xb{b}{kh}", name=f"xb{b}{kh}")
               for kh in range(2)] for b in range(2)]
        hfT = pp.tile([128, NQ], BF, tag="hfT", name="hfT")
        hbT = pp.tile([128, HBT_W], BF, tag="hbT", name="hbT")
        hf_head = pp.tile([128, 64], BF, tag="hfh", name="hfh")
        hb_head = pp.tile([128, 64], BF, tag="hbh", name="hbh")
        # contiguous recurrent h state (fast writes on the critical chain):
        # hcurF col = 2*lane + (s%2)  (u32-pairable for the hfT scatter)
        # hcurB col = 128*(s%2) + lane
        hcurF = pp.tile([128, 256], BF, tag="hcurF", name="hcurF")
        hcurB = pp.tile([128, 256], BF, tag="hcurB", name="hcurB")
        wih = [[pp.tile([128, 512], BF, tag=f"wih{d}{k}", name=f"wih{d}{k}") for k in range(2)]
               for d in range(2)]
        whh = [pp.tile([128, 512], BF, tag=f"whh{d}", name=f"whh{d}") for d in range(2)]
        b48 = pp.tile([128, 128], BF, tag="b48", name="b48")
        i48 = pp.tile([128, 1024], BF, tag="i48", name="i48")
        h0 = [pp.tile([128, 128], BF, tag=f"h0{d}", name=f"h0{d}") for d in range(2)]
        c0 = [pp.tile([128, 128], BF, tag=f"c0{d}", name=f"c0{d}") for d in range(2)]
        wom = [pp.tile([128, 256], BF, tag=f"wom{k}", name=f"wom{k}") for k in range(2)]
        uo = [pp.tile([128, 1], BF, tag=f"uo{k}", name=f"uo{k}") for k in range(2)]
        iota_t = pp.tile([128, SWIN], BF, tag="iota", name="iota")
        identb = pp.tile([128, 128], BF, tag="identb", name="identb")
        identf = pp.tile([128, 128], F32, tag="identf", name="identf")
        seg_t = pp.tile([128, 129], F32, tag="seg", name="seg")
        CFB = pp.tile([128, 256], BF, tag="CFB", name="CFB")
        e_cm = pp.tile([128, 129], F32, tag="ecm", name="ecm")
        hfx = pp.tile([128, 128], BF, tag="hfx", name="hfx")
        hbx = pp.tile([128, 128], BF, tag="hbx", name="hbx")
        ctx_sb = [pp.tile([128 if k == 0 else SWIN - 128, 257], F32,
                          tag=f"ctxsb{k}", name=f"ctxsb{k}") for k in range(2)]

        # ---- input DMAs ----
        def dma_xchunk(c):
            # stage chunk c (steps [CH*c, CH*c+CH)) of the gathered x
            w = 128 * CH
            if c * CH < NSF:
                wf = min(w, 128 * NSF - c * w)
                for kh in range(2):
                    nc.sync.dma_start(xf[c % 2][kh][:, 0:wf],
                                      xsf_in[128 * kh:128 * kh + 128,
                                             c * w:c * w + wf])
            if c * CH < NSB:
                wb = min(w, 128 * NSB - c * w)
                for kh in range(2):
                    nc.sync.dma_start(xb[c % 2][kh][:, 0:wb],
                                      xsb_in[128 * kh:128 * kh + 128,
                                             c * w:c * w + wb])

        # LSTM-critical tensors first so compute starts ASAP; attention-only
        # tensors and the second x-chunk trail behind.
        nc.sync.dma_start(b48[:], b48_in[:])
        nc.sync.dma_start(i48[:], i48_in[:])
        for d, t_ in ((0, wih_f_in), (1, wih_b_in)):
            nc.sync.dma_start(wih[d][0][:], t_[0:128, :])
            nc.sync.dma_start(wih[d][1][:], t_[128:256, :])
        nc.sync.dma_start(whh[0][:], whh_f_in[:])
        nc.sync.dma_start(whh[1][:], whh_b_in[:])
        nc.sync.dma_start(h0[0][:], h0f_in[:])
        nc.sync.dma_start(c0[0][:], c0f_in[:])
        nc.sync.dma_start(h0[1][:], h0b_in[:])
        nc.sync.dma_start(c0[1][:], c0b_in[:])
        dma_xchunk(0)
        dma_xchunk(1)
        nc.sync.dma_start(wom[0][:], wom_in[0:128, :])
        nc.sync.dma_start(wom[1][:], wom_in[128:256, :])
        nc.sync.dma_start(uo[0][:], uo_in[0:128, :])
        nc.sync.dma_start(uo[1][:], uo_in[128:256, :])
        nc.sync.dma_start(iota_t[:], iota_in[:])
        nc.sync.dma_start(identb[:], identb_in[:])
        nc.sync.dma_start(identf[:], identf_in[:])
        nc.sync.dma_start(seg_t[:], seg_in[:])

        # init cell state from seeds: CFB = [c0f | c0b]
        nc.vector.tensor_copy(CFB[:, 0:128], c0[0][:])
        nc.vector.tensor_copy(CFB[:, 128:256], c0[1][:])

        def h_src(d, s):
            # h state produced at step s-1 (read at step s)
            if s == 0:
                return h0[d][:]
            p = (s - 1) % 2
            if d == 0:
                return hcurF[:, p:p + 2 * 127 + 1:2]
            return hcurB[:, 128 * p:128 * p + 128]

        def h_dst(d, s):
            p = s % 2
            if d == 0:
                return hcurF[:, p:p + 2 * 127 + 1:2]
            return hcurB[:, 128 * p:128 * p + 128]

        with tc.tile_pool(name="psG", bufs=4, space="PSUM") as psg, \
             tc.tile_pool(name="sig", bufs=3) as sigp, \
             tc.tile_pool(name="tg", bufs=3) as tgp, \
             tc.tile_pool(name="tcn", bufs=3) as tcp, \
             tc.tile_pool(name="tmp1", bufs=3) as t1p, \
             tc.tile_pool(name="tmp2", bufs=3) as t2p:

            G = {}

            def emit_pre(s_):
                # pre-gate + bias matmuls for step s_ (both dirs)
                if s_ >= NSB:
                    return
                g = psg.tile([128, 1024], F32, tag="G", name="G")
                G[s_] = g
                for hb in range(2):
                    nc.tensor.matmul(g[:, 512 * hb:512 * hb + 512], b48[:],
                                     i48[:, 512 * hb:512 * hb + 512],
                                     start=True, stop=False)
                o = 128 * (s_ % CH)
                cb = (s_ // CH) % 2
                for d in range(2):
                    if d == 0 and s_ >= NSF:
                        continue
                    xt = xf[cb] if d == 0 else xb[cb]
                    for kh in range(2):
                        rhs = xt[kh][:, o:o + 128]
                        for j in range(4):
                            nc.tensor.matmul(
                                g[:, 256 * j + 128 * d:256 * j + 128 * d + 128],
                                wih[d][kh][:, 128 * j:128 * j + 128],
                                rhs, start=False, stop=False)

            for s_ in range(3):
                emit_pre(s_)

            for s in range(NSB):
                if s % CH == 0 and s >= CH and (s // CH + 1) * CH < NSB:
                    dma_xchunk(s // CH + 1)
                g = G.pop(s)
                emit_pre(s + 3)
                # w_hh matmuls (accumulate into this step's gate region)
                for d in range(2):
                    if d == 0 and s >= NSF:
                        continue
                    hs = h_src(d, s)
                    for j in range(4):
                        nc.tensor.matmul(
                            g[:, 256 * j + 128 * d:256 * j + 128 * d + 128],
                            whh[d][:, 128 * j:128 * j + 128], hs,
                            start=False, stop=True)
                # gates, split per direction so the F/B chains pipeline.
                # gate-major layout: g cols 256*j + 128*d, j in (i,f,o,g)
                dirs = [0, 1] if s < NSF else [1]
                sig = sigp.tile([128, 768], BF, tag="sig", name="sig")
                tg = tgp.tile([128, 256], BF, tag="tg", name="tg")
                t1 = t1p.tile([128, 256], BF, tag="t1", name="t1")
                t2 = t2p.tile([128, 256], BF, tag="t2", name="t2")
                tcn = tcp.tile([128, 256], BF, tag="tcn", name="tcn")

                for d in dirs:
                    # sigmoid over the i,f,o blocks of dir d: AP [j:3][128]
                    gv = g[:].rearrange("p (j q) -> p j q", q=256)[
                        :, 0:3, 128 * d:128 * d + 128]
                    sv = sig[:].rearrange("p (j q) -> p j q", q=256)[
                        :, 0:3, 128 * d:128 * d + 128]
                    nc.scalar.activation(sv, gv, AF.Sigmoid)
                for d in dirs:
                    nc.scalar.activation(tg[:, 128 * d:128 * d + 128],
                                         g[:, 768 + 128 * d:896 + 128 * d],
                                         AF.Tanh)
                for d in dirs:
                    nc.vector.tensor_tensor(t1[:, 128 * d:128 * d + 128],
                                            sig[:, 256 + 128 * d:384 + 128 * d],
                                            CFB[:, 128 * d:128 * d + 128],
                                            ALU.mult)
                for d in dirs:
                    nc.vector.tensor_tensor(t2[:, 128 * d:128 * d + 128],
                                            sig[:, 128 * d:128 * d + 128],
                                            tg[:, 128 * d:128 * d + 128],
                                            ALU.mult)
                    nc.vector.tensor_tensor(CFB[:, 128 * d:128 * d + 128],
                                            t1[:, 128 * d:128 * d + 128],
                                            t2[:, 128 * d:128 * d + 128],
                                            ALU.add)
                for d in dirs:
                    nc.scalar.activation(tcn[:, 128 * d:128 * d + 128],
                                         CFB[:, 128 * d:128 * d + 128], AF.Tanh)
                    nc.vector.tensor_tensor(h_dst(d, s),
                                            sig[:, 512 + 128 * d:640 + 128 * d],
                                            tcn[:, 128 * d:128 * d + 128],
                                            ALU.mult)
                if s < B:
                    nc.vector.tensor_copy(hf_head[:, s:s + 1],
                                          hcurF[:, s % 2:s % 2 + 1])
                    nc.vector.tensor_copy(hb_head[:, 63 - s:64 - s],
                                          hcurB[:, 128 * (s % 2) + 126:
                                                128 * (s % 2) + 127])
                # off-chain scatters into token-major hfT/hbT
                if s >= B and s < NSF and s % 2 == 1:
                    # fwd pair (s-1, s): u32 packed copy, dst stride 64 u32
                    c0u = (s - 1 - 64) // 2
                    nc.vector.tensor_copy(
                        hfT[:].bitcast(mybir.dt.uint32)[
                            :, c0u:c0u + 64 * 127 + 1:64],
                        hcurF[:].bitcast(mybir.dt.uint32))
                if s >= B:
                    a = 192 - s
                    nc.gpsimd.tensor_copy(hbT[:, a:a + 129 * 127 + 1:129],
                                          hcurB[:, 128 * (s % 2):
                                                128 * (s % 2) + 128])

        # ---------------- attention phase ----------------
        # assemble extra window tiles
        nc.vector.tensor_copy(hfx[:, 0:64], hf_head[:])
        nc.vector.tensor_copy(hfx[:, 64:128], hfT[:, 16256:16320])
        nc.vector.tensor_copy(hbx[:, 0:64], hbT[:, 63:127])
        nc.vector.tensor_copy(hbx[:, 64:128], hb_head[:])

        # merged attention pipeline: per 512-token group, emit u/att matmuls,
        # stage e via a DRAM round-trip + [4,128] transpose, then the 4 ctx
        # tiles (h transposes + e-weighted one-hot accumulation).
        with tc.tile_pool(name="psU", bufs=2, space="PSUM") as psu, \
             tc.tile_pool(name="uT", bufs=3) as utp, \
             tc.tile_pool(name="psA", bufs=1, space="PSUM") as psa, \
             tc.tile_pool(name="anm", bufs=2) as anmp, \
             tc.tile_pool(name="psE", bufs=1, space="PSUM") as pse, \
             tc.tile_pool(name="psT2", bufs=2, space="PSUM") as pst2, \
             tc.tile_pool(name="yp", bufs=2) as yp, \
             tc.tile_pool(name="iw", bufs=2) as iwp, \
             tc.tile_pool(name="psC", bufs=1, space="PSUM") as psc:
            ctxp = [psc.tile([128 if k == 0 else SWIN - 128, 257], F32,
                             tag=f"ctxp{k}", name=f"ctxp{k}") for k in range(2)]

            def emit_u_att(gidx):
                if gidx < 32:
                    n = 512
                    hfr = hfT[:, 512 * gidx:512 * gidx + 512]
                    hbr = hbT[:, 512 * gidx + 127:512 * gidx + 127 + 512]
                    aout = att_dram[0:1, 512 * gidx:512 * gidx + 512]
                else:
                    n = 128
                    hfr = hfx[:]
                    hbr = hbx[:]
                    aout = att_dram[0:1, NQ:NQ + 128]
                pa = psa.tile([1, 512], F32, tag="psA", name="psA")
                for c2 in range(2):
                    pu = psu.tile([128, 512], F32, tag="psU", name="psU")
                    nc.tensor.matmul(pu[:, 0:n], wom[0][:, 128 * c2:128 * c2 + 128],
                                     hfr, start=True, stop=False)
                    nc.tensor.matmul(pu[:, 0:n], wom[1][:, 128 * c2:128 * c2 + 128],
                                     hbr, start=False, stop=True)
                    ut = utp.tile([128, 512], BF, tag="uT", name="uT")
                    nc.scalar.activation(ut[:, 0:n], pu[:, 0:n], AF.Tanh)
                    nc.tensor.matmul(pa[0:1, 0:n], uo[c2][:], ut[:, 0:n],
                                     start=(c2 == 0), stop=(c2 == 1))
                asb = utp.tile([1, 512], F32, tag="asb", name="asb")
                nc.vector.tensor_copy(asb[0:1, 0:n], pa[0:1, 0:n])
                nc.sync.dma_start(aout, asb[0:1, 0:n])

            def emit_ctx_tile(nti, hfr, hbr):
                ps_t = pst2.tile([128, 256], BF, tag="psT2", name="psT2")
                nc.tensor.transpose(ps_t[:, 0:128], hfr, identb[:])
                nc.tensor.transpose(ps_t[:, 128:256], hbr, identb[:])
                y = yp.tile([128, 257], BF, tag="y", name="y")
                if nti % 2 == 0:
                    nc.scalar.copy(y[:, 0:256], ps_t[:])
                else:
                    nc.vector.tensor_copy(y[:, 0:256], ps_t[:])
                if nti < 2:
                    nc.vector.memset(y[:, 256:257], 1.0)
                iw = iwp.tile([128, SWIN], BF, tag="iw", name="iw")
                nc.vector.tensor_scalar(iw[:], iota_t[:],
                                        seg_t[:, nti:nti + 1],
                                        e_cm[:, nti:nti + 1],
                                        ALU.is_equal, ALU.mult)
                for k in range(2):
                    nc.tensor.matmul(ctxp[k][:], iw[:, 128 * k:SWIN if k else 128],
                                     y[:], start=(nti == 0), stop=(nti == NTILE))

            for gidx in range(33):
                emit_u_att(gidx)
                if gidx < 32:
                    a4 = anmp.tile([4, 128], F32, tag="a4", name="a4")
                    nc.sync.dma_start(
                        a4[:], att_dram[0:1, 512 * gidx:512 * gidx + 512]
                        .rearrange("a (n p) -> (a n) p", p=128))
                    pe4 = pse.tile([128, 4], F32, tag="pe4", name="pe4")
                    nc.tensor.transpose(pe4[:], a4[:], identf[0:4, 0:4])
                    nc.scalar.activation(e_cm[:, 4 * gidx:4 * gidx + 4], pe4[:],
                                         AF.Exp)
                    for t4 in range(4):
                        nti = 4 * gidx + t4
                        emit_ctx_tile(nti, hfT[:, 128 * nti:128 * nti + 128],
                                      hbT[:, 128 * nti + 127:128 * nti + 255])
                else:
                    att_x = anmp.tile([128, 1], F32, tag="attx", name="attx")
                    nc.sync.dma_start(
                        att_x[:],
                        att_dram[0:1, NQ:NQ + 128].rearrange(
                            "a (n p) -> (a n) p", p=1))
                    nc.scalar.activation(e_cm[:, 128:129], att_x[:], AF.Exp)
                    emit_ctx_tile(NTILE, hfx[:], hbx[:])
            for k in range(2):
                nc.vector.tensor_copy(ctx_sb[k][:], ctxp[k][:])
        for k in range(2):
            nc.sync.dma_start(ctx_out[128 * k:128 * k + (SWIN - 128 if k else 128),
                                      :], ctx_sb[k][:])

    nc.finalize()
    _BUILT["nc"] = nc
    return nc


def _host_prep(inputs):
    x = np.asarray(inputs["sentence"], np.float32)
    doc_mask = np.asarray(inputs["doc_mask"]).astype(np.int64)
    h0g = np.asarray(inputs["h0"], np.float32)
    c0g = np.asarray(inputs["c0"], np.float32)

    perm = np.r_[0:128, 128:256, 384:512, 256:384]  # i,f,o,g order

    def wprep(w):  # [4H, X] -> lhsT [X, 4H] with gate perm, bf16
        return np.ascontiguousarray(w.astype(np.float32).T[:, perm]).astype(BF16)

    wih = {d: wprep(np.asarray(inputs[f"w_ih_{s}"], np.float32))
           for d, s in ((0, "f"), (1, "b"))}
    whh = {d: wprep(np.asarray(inputs[f"w_hh_{s}"], np.float32))
           for d, s in ((0, "f"), (1, "b"))}
    bias = {d: (np.asarray(inputs[f"b_ih_{s}"], np.float32)
                + np.asarray(inputs[f"b_hh_{s}"], np.float32))[perm]
            for d, s in ((0, "f"), (1, "b"))}
    b48 = np.zeros((128, 128), np.float32)
    for d in range(2):
        for k in range(4):
            b48[2 * k + d, :] = bias[d][128 * k:128 * k + 128]
    b48 = b48.astype(BF16)
    i48 = np.zeros((128, 1024), np.float32)
    for r in range(8):
        i48[r, 128 * r:128 * r + 128] = 1.0
    i48 = i48.astype(BF16)

    NSF_, NSB_ = 192, 193
    idxf = (64 + np.arange(NSF_)[:, None] + 128 * np.arange(128)[None, :])
    idxb = (193 - np.arange(NSB_)[:, None] + 129 * np.arange(128)[None, :])

    wom = np.asarray(inputs["w_omega"], np.float32).astype(BF16)
    uo = np.asarray(inputs["u_omega"], np.float32).astype(BF16)
    iota = np.tile(np.arange(SWIN, dtype=np.float32), (128, 1)).astype(BF16)
    identb = np.eye(128, dtype=np.float32).astype(BF16)
    identf = np.eye(128, dtype=np.float32)

    seg_global = np.searchsorted(doc_mask, np.arange(T), side="right")

    in_maps = []
    s_los = []
    xpad = np.zeros((T + 512, D), np.float32)
    xpad[64:64 + T] = x  # global row r ↔ token r - 64
    for c in range(NCORE):
        tc0 = c * PC
        xs = xpad[tc0:tc0 + SH]  # token tc0-64+i at row i
        # step-major gathers: col 128*s + lane
        xsf = np.ascontiguousarray(
            xs[idxf.reshape(-1)].T).astype(BF16)   # [256, 128*NSF]
        xsb = np.ascontiguousarray(
            xs[idxb.reshape(-1)].T).astype(BF16)   # [256, 128*NSB]

        # seeds
        h0f = np.zeros((128, 128), np.float32)
        c0f = np.zeros((128, 128), np.float32)
        h0b = np.zeros((128, 128), np.float32)
        c0b = np.zeros((128, 128), np.float32)
        if c == 0:
            h0f[:, 0] = h0g[0]
            c0f[:, 0] = c0g[0]
        if c == NCORE - 1:
            h0b[:, 126] = h0g[1]
            c0b[:, 126] = c0g[1]

        # segment ids, col-major [128, 129]
        segm = np.full((128, 129), -1.0, np.float32)
        toks_main = tc0 + 64 + np.arange(NQ)
        valid = toks_main < T
        if c == NCORE - 1:
            valid &= (np.arange(NQ) < 16256)  # tail handled by W_tail
        toks_extra = np.full(128, -1, np.int64)
        if c == 0:
            toks_extra[0:64] = np.arange(64)          # W_head: tokens [0,64)
        if c == NCORE - 1:
            toks_extra[64:128] = T - 64 + np.arange(64)  # W_tail
        all_toks = np.concatenate([toks_main[valid],
                                   toks_extra[toks_extra >= 0]])
        s_lo = int(seg_global[all_toks].min()) if all_toks.size else 0
        s_hi = int(seg_global[all_toks].max()) if all_toks.size else 0
        assert s_hi - s_lo < SWIN, f"segment window too wide: {s_hi - s_lo}"
        s_los.append(s_lo)
        sm = np.where(valid, seg_global[np.minimum(toks_main, T - 1)] - s_lo,
                      -1.0).astype(np.float32)
        segm[:, 0:128] = sm.reshape(128, 128).T  # segm[p, n] = seg(q=128n+p)
        se = np.full(128, -1.0, np.float32)
        mask_x = toks_extra >= 0
        se[mask_x] = seg_global[toks_extra[mask_x]] - s_lo
        segm[:, 128] = se

        in_maps.append({
            "xSf": xsf, "xSb": xsb,
            "wih_f": wih[0], "wih_b": wih[1],
            "whh_f": whh[0], "whh_b": whh[1],
            "b48": b48, "i48": i48,
            "h0f": h0f.astype(BF16), "c0f": c0f.astype(BF16),
            "h0b": h0b.astype(BF16), "c0b": c0b.astype(BF16),
            "wom": wom, "uo": uo, "iota": iota,
            "identb": identb, "identf": identf,
            "seg": segm,
        })
    return in_maps, s_los


def kernel(**inputs):
    global LAST_RESULT
    from concourse.bass_utils import run_bass_kernel_spmd

    nc = _build()
    in_maps, s_los = _host_prep(inputs)
    res = run_bass_kernel_spmd(nc, in_maps, core_ids=list(range(NCORE)))
    LAST_RESULT = res

    G = np.zeros((S + SWIN, 257), np.float64)
    for c in range(NCORE):
        ctx = np.asarray(res.results[c]["ctx"], np.float32)
        G[s_los[c]:s_los[c] + SWIN] += ctx
    G = G[:S]
    z = G[:, 256]
    ctx = G[:, :256] / np.where(z == 0, 1.0, z)[:, None]
    w_tag = np.asarray(inputs["w_tag"], np.float32)
    b_tag = np.asarray(inputs["b_tag"], np.float32)
    out = ctx.astype(np.float32) @ w_tag.T + b_tag
    return out.astype(np.float32)



# revision 41
# speedup vs baseline: 1.2967x; 1.2967x over previous
"""Bass/Trainium2 kernel for nn_BiLSTM_Tok_83837761618147.

Strategy (8 NeuronCores, SPMD, full inputs in / full output out):
  - Token dim sharded 8 ways (16384 tokens/core, with halos).
  - BiLSTM parallelized via chunked recurrence with burn-in: each core runs
    128 lanes x (128+64) steps forward and 128 lanes x (129+64) steps
    backward (state forgets exponentially; 64 warmup steps reach fp32
    accuracy; the true h0/c0-seeded lanes cover the sequence ends exactly).
  - Gate pre-activations computed by PE matmuls directly into PSUM
    (bias via a K=4 indicator matmul); w_hh @ h accumulated on top.
  - Attention (tanh/logits/exp) + ragged segment softmax-sum done on
    device via an e-weighted one-hot (token x segment-window) matmul.
  - Host combines per-core partial [segment, 257] sums, normalizes, and
    applies the tiny tag projection.
"""

import numpy as np
import ml_dtypes

BF16 = ml_dtypes.bfloat16

T = 131072
D = 256
H = 128
HID = 256
TAGS = 10
S = 1024
NCORE = 8
PC = T // NCORE          # 16384 tokens per core
B = 64                   # burn-in steps
LF = 128                 # forward lane length (tokens per lane)
LB = 129                 # backward lane length
NL = 128                 # lanes per direction
NSF = B + LF             # 192 forward steps
NSB = B + LB             # 193 backward steps
SH = 16640               # x shard rows [tc0-64, tc0-64+SH)
SWIN = 192               # segment window width per core (max observed 142)
NQ = PC                  # main attention window positions
NTILE = NQ // 128        # 128 main token tiles
HBW = LB * NL - LB + LB + B  # hbT width: 16512
HBT_W = 16512
ATT_W = NQ + 128         # att buffer width (main + extra tile)

_BUILT = {}
LAST_RESULT = None


def _build():
    if "nc" in _BUILT:
        return _BUILT["nc"]
    import contextlib
    from concourse import bacc, mybir
    from concourse.tile import TileContext

    F32 = mybir.dt.float32
    BF = mybir.dt.bfloat16
    AF = mybir.ActivationFunctionType
    ALU = mybir.AluOpType

    nc = bacc.Bacc()

    def din(name, shape, dt):
        return nc.declare_dram_parameter(name, list(shape), dt, isOutput=False)

    xsf_in = din("xSf", [256, 128 * NSF], BF)
    xsb_in = din("xSb", [256, 128 * NSB], BF)
    wih_f_in = din("wih_f", [256, 512], BF)
    wih_b_in = din("wih_b", [256, 512], BF)
    whh_f_in = din("whh_f", [128, 512], BF)
    whh_b_in = din("whh_b", [128, 512], BF)
    b48_in = din("b48", [128, 128], BF)
    i48_in = din("i48", [128, 1024], BF)
    h0f_in = din("h0f", [128, 128], BF)
    c0f_in = din("c0f", [128, 128], BF)
    h0b_in = din("h0b", [128, 128], BF)
    c0b_in = din("c0b", [128, 128], BF)
    wom_in = din("wom", [256, 256], BF)
    uo_in = din("uo", [256, 1], BF)
    iota_in = din("iota", [128, SWIN], BF)
    identb_in = din("identb", [128, 128], BF)
    identf_in = din("identf", [128, 128], F32)
    seg_in = din("seg", [128, 129], F32)
    ctx_out = nc.declare_dram_parameter("ctx", [SWIN, 257], F32, isOutput=True)
    att_dram = nc.dram_tensor("att_stage", [1, ATT_W], F32)

    with TileContext(nc) as tc, contextlib.ExitStack() as ctx:
        pp = ctx.enter_context(tc.tile_pool(name="persist", bufs=1))

        # chunked step-major x staging (CH steps per chunk, double buffered)
        CH = 16
        xf = [[pp.tile([128, 128 * CH], BF, tag=f"xf{b}{kh}", name=f"xf{b}{kh}")
               for kh in range(2)] for b in range(2)]
        xb = [[pp.tile([128, 128 * CH], BF, tag=f"xb{b}{kh}", name=f"xb{b}{kh}")
               for kh in range(2)] for b in range(2)]
        hfT = pp.tile([128, NQ], BF, tag="hfT", name="hfT")
        hbT = pp.tile([128, HBT_W], BF, tag="hbT", name="hbT")
        hf_head = pp.tile([128, 64], BF, tag="hfh", name="hfh")
        hb_head = pp.tile([128, 64], BF, tag="hbh", name="hbh")
        # contiguous recurrent h state (fast writes on the critical chain):
        # hcurF col = 2*lane + (s%2)  (u32-pairable for the hfT scatter)
        # hcurB col = 128*(s%2) + lane
        hcurF = pp.tile([128, 256], BF, tag="hcurF", name="hcurF")
        hcurB = pp.tile([128, 256], BF, tag="hcurB", name="hcurB")
        wih = [[pp.tile([128, 512], BF, tag=f"wih{d}{k}", name=f"wih{d}{k}") for k in range(2)]
               for d in range(2)]
        whh = [pp.tile([128, 512], BF, tag=f"whh{d}", name=f"whh{d}") for d in range(2)]
        b48 = pp.tile([128, 128], BF, tag="b48", name="b48")
        i48 = pp.tile([128, 1024], BF, tag="i48", name="i48")
        h0 = [pp.tile([128, 128], BF, tag=f"h0{d}", name=f"h0{d}") for d in range(2)]
        c0 = [pp.tile([128, 128], BF, tag=f"c0{d}", name=f"c0{d}") for d in range(2)]
        wom = [pp.tile([128, 256], BF, tag=f"wom{k}", name=f"wom{k}") for k in range(2)]
        uo = [pp.tile([128, 1], BF, tag=f"uo{k}", name=f"uo{k}") for k in range(2)]
        iota_t = pp.tile([128, SWIN], BF, tag="iota", name="iota")
        identb = pp.tile([128, 128], BF, tag="identb", name="identb")
        identf = pp.tile([128, 128], F32, tag="identf", name="identf")
        seg_t = pp.tile([128, 129], F32, tag="seg", name="seg")
        CFB = pp.tile([128, 256], BF, tag="CFB", name="CFB")
        e_cm = pp.tile([128, 129], F32, tag="ecm", name="ecm")
        hfx = pp.tile([128, 128], BF, tag="hfx", name="hfx")
        hbx = pp.tile([128, 128], BF, tag="hbx", name="hbx")
        ctx_sb = [pp.tile([128 if k == 0 else SWIN - 128, 257], F32,
                          tag=f"ctxsb{k}", name=f"ctxsb{k}") for k in range(2)]

        # ---- input DMAs ----
        def dma_xchunk(c):
            # stage chunk c (steps [CH*c, CH*c+CH)) of the gathered x
            w = 128 * CH
            if c * CH < NSF:
                wf = min(w, 128 * NSF - c * w)
                for kh in range(2):
                    nc.sync.dma_start(xf[c % 2][kh][:, 0:wf],
                                      xsf_in[128 * kh:128 * kh + 128,
                                             c * w:c * w + wf])
            if c * CH < NSB:
                wb = min(w, 128 * NSB - c * w)
                for kh in range(2):
                    nc.sync.dma_start(xb[c % 2][kh][:, 0:wb],
                                      xsb_in[128 * kh:128 * kh + 128,
                                             c * w:c * w + wb])

        nc.sync.dma_start(b48[:], b48_in[:])
        nc.sync.dma_start(i48[:], i48_in[:])
        for d, t_ in ((0, wih_f_in), (1, wih_b_in)):
            nc.sync.dma_start(wih[d][0][:], t_[0:128, :])
            nc.sync.dma_start(wih[d][1][:], t_[128:256, :])
        nc.sync.dma_start(whh[0][:], whh_f_in[:])
        nc.sync.dma_start(whh[1][:], whh_b_in[:])
        nc.sync.dma_start(h0[0][:], h0f_in[:])
        nc.sync.dma_start(c0[0][:], c0f_in[:])
        nc.sync.dma_start(h0[1][:], h0b_in[:])
        nc.sync.dma_start(c0[1][:], c0b_in[:])
        dma_xchunk(0)
        dma_xchunk(1)
        nc.sync.dma_start(wom[0][:], wom_in[0:128, :])
        nc.sync.dma_start(wom[1][:], wom_in[128:256, :])
        nc.sync.dma_start(uo[0][:], uo_in[0:128, :])
        nc.sync.dma_start(uo[1][:], uo_in[128:256, :])
        nc.sync.dma_start(iota_t[:], iota_in[:])
        nc.sync.dma_start(identb[:], identb_in[:])
        nc.sync.dma_start(identf[:], identf_in[:])
        nc.sync.dma_start(seg_t[:], seg_in[:])

        # init cell state from seeds: CFB = [c0f | c0b]
        nc.vector.tensor_copy(CFB[:, 0:128], c0[0][:])
        nc.vector.tensor_copy(CFB[:, 128:256], c0[1][:])

        def h_src(d, s):
            # h state produced at step s-1 (read at step s)
            if s == 0:
                return h0[d][:]
            p = (s - 1) % 2
            if d == 0:
                return hcurF[:, p:p + 2 * 127 + 1:2]
            return hcurB[:, 128 * p:128 * p + 128]

        def h_dst(d, s):
            p = s % 2
            if d == 0:
                return hcurF[:, p:p + 2 * 127 + 1:2]
            return hcurB[:, 128 * p:128 * p + 128]

        with tc.tile_pool(name="psG", bufs=4, space="PSUM") as psg, \
             tc.tile_pool(name="sig", bufs=3) as sigp, \
             tc.tile_pool(name="tg", bufs=3) as tgp, \
             tc.tile_pool(name="tcn", bufs=3) as tcp, \
             tc.tile_pool(name="tmp1", bufs=3) as t1p, \
             tc.tile_pool(name="tmp2", bufs=3) as t2p:

            G = {}

            def emit_pre(s_):
                # pre-gate + bias matmuls for step s_ (both dirs)
                if s_ >= NSB:
                    return
                g = psg.tile([128, 1024], F32, tag="G", name="G")
                G[s_] = g
                for hb in range(2):
                    nc.tensor.matmul(g[:, 512 * hb:512 * hb + 512], b48[:],
                                     i48[:, 512 * hb:512 * hb + 512],
                                     start=True, stop=False)
                o = 128 * (s_ % CH)
                cb = (s_ // CH) % 2
                for d in range(2):
                    if d == 0 and s_ >= NSF:
                        continue
                    xt = xf[cb] if d == 0 else xb[cb]
                    for kh in range(2):
                        rhs = xt[kh][:, o:o + 128]
                        for j in range(4):
                            nc.tensor.matmul(
                                g[:, 256 * j + 128 * d:256 * j + 128 * d + 128],
                                wih[d][kh][:, 128 * j:128 * j + 128],
                                rhs, start=False, stop=False)

            for s_ in range(3):
                emit_pre(s_)

            for s in range(NSB):
                if s % CH == 0 and s >= CH and (s // CH + 1) * CH < NSB:
                    dma_xchunk(s // CH + 1)
                g = G.pop(s)
                emit_pre(s + 3)
                # w_hh matmuls (accumulate into this step's gate region)
                for d in range(2):
                    if d == 0 and s >= NSF:
                        continue
                    hs = h_src(d, s)
                    for j in (0, 1, 3, 2):  # i,f first; g before o
                        nc.tensor.matmul(
                            g[:, 256 * j + 128 * d:256 * j + 128 * d + 128],
                            whh[d][:, 128 * j:128 * j + 128], hs,
                            start=False, stop=True)
                # gates (gate-major layout: cols 256*j + 128*d, j in i,f,o,g)
                # sigmoid split: i,f first (feeds t1/t2, ready after 6 of the
                # 8 whh matmuls), o later (only needed for h at chain end)
                sig = sigp.tile([128, 768], BF, tag="sig", name="sig")
                nc.scalar.activation(sig[:, 0:512], g[:, 0:512], AF.Sigmoid)
                tg = tgp.tile([128, 256], BF, tag="tg", name="tg")
                nc.scalar.activation(tg[:], g[:, 768:1024], AF.Tanh)
                nc.scalar.activation(sig[:, 512:768], g[:, 512:768], AF.Sigmoid)
                # c update (all contiguous [128, 256] = [fwd | bwd])
                t1 = t1p.tile([128, 256], BF, tag="t1", name="t1")
                t2 = t2p.tile([128, 256], BF, tag="t2", name="t2")
                nc.vector.tensor_tensor(t1[:], sig[:, 256:512], CFB[:], ALU.mult)
                nc.vector.tensor_tensor(t2[:], sig[:, 0:256], tg[:], ALU.mult)
                nc.vector.tensor_tensor(CFB[:], t1[:], t2[:], ALU.add)
                tcn = tcp.tile([128, 256], BF, tag="tcn", name="tcn")
                nc.scalar.activation(tcn[:], CFB[:], AF.Tanh)
                # h = sigma_o * tanh(c) into the contiguous recurrent slot
                for d in range(2):
                    if d == 0 and s >= NSF:
                        continue
                    nc.vector.tensor_tensor(h_dst(d, s),
                                            sig[:, 512 + 128 * d:640 + 128 * d],
                                            tcn[:, 128 * d:128 * d + 128],
                                            ALU.mult)
                if s < B:
                    nc.vector.tensor_copy(hf_head[:, s:s + 1],
                                          hcurF[:, s % 2:s % 2 + 1])
                    nc.vector.tensor_copy(hb_head[:, 63 - s:64 - s],
                                          hcurB[:, 128 * (s % 2) + 126:
                                                128 * (s % 2) + 127])
                # off-chain scatters into token-major hfT/hbT
                if s >= B and s < NSF and s % 2 == 1:
                    # fwd pair (s-1, s): u32 packed copy, dst stride 64 u32
                    c0u = (s - 1 - 64) // 2
                    nc.vector.tensor_copy(
                        hfT[:].bitcast(mybir.dt.uint32)[
                            :, c0u:c0u + 64 * 127 + 1:64],
                        hcurF[:].bitcast(mybir.dt.uint32))
                if s >= B:
                    a = 192 - s
                    nc.gpsimd.tensor_copy(hbT[:, a:a + 129 * 127 + 1:129],
                                          hcurB[:, 128 * (s % 2):
                                                128 * (s % 2) + 128])

        # ---------------- attention phase ----------------
        # assemble extra window tiles
        nc.vector.tensor_copy(hfx[:, 0:64], hf_head[:])
        nc.vector.tensor_copy(hfx[:, 64:128], hfT[:, 16256:16320])
        nc.vector.tensor_copy(hbx[:, 0:64], hbT[:, 63:127])
        nc.vector.tensor_copy(hbx[:, 64:128], hb_head[:])

        # merged attention pipeline: per 512-token group, emit u/att matmuls,
        # stage e via a DRAM round-trip + [4,128] transpose, then the 4 ctx
        # tiles (h transposes + e-weighted one-hot accumulation).
        with tc.tile_pool(name="psU", bufs=2, space="PSUM") as psu, \
             tc.tile_pool(name="uT", bufs=3) as utp, \
             tc.tile_pool(name="psA", bufs=1, space="PSUM") as psa, \
             tc.tile_pool(name="anm", bufs=2) as anmp, \
             tc.tile_pool(name="psE", bufs=1, space="PSUM") as pse, \
             tc.tile_pool(name="psT2", bufs=2, space="PSUM") as pst2, \
             tc.tile_pool(name="yp", bufs=2) as yp, \
             tc.tile_pool(name="iw", bufs=2) as iwp, \
             tc.tile_pool(name="psC", bufs=1, space="PSUM") as psc:
            ctxp = [psc.tile([128 if k == 0 else SWIN - 128, 257], F32,
                             tag=f"ctxp{k}", name=f"ctxp{k}") for k in range(2)]

            def emit_u_att(gidx):
                if gidx < 32:
                    n = 512
                    hfr = hfT[:, 512 * gidx:512 * gidx + 512]
                    hbr = hbT[:, 512 * gidx + 127:512 * gidx + 127 + 512]
                    aout = att_dram[0:1, 512 * gidx:512 * gidx + 512]
                else:
                    n = 128
                    hfr = hfx[:]
                    hbr = hbx[:]
                    aout = att_dram[0:1, NQ:NQ + 128]
                pa = psa.tile([1, 512], F32, tag="psA", name="psA")
                for c2 in range(2):
                    pu = psu.tile([128, 512], F32, tag="psU", name="psU")
                    nc.tensor.matmul(pu[:, 0:n], wom[0][:, 128 * c2:128 * c2 + 128],
                                     hfr, start=True, stop=False)
                    nc.tensor.matmul(pu[:, 0:n], wom[1][:, 128 * c2:128 * c2 + 128],
                                     hbr, start=False, stop=True)
                    ut = utp.tile([128, 512], BF, tag="uT", name="uT")
                    nc.scalar.activation(ut[:, 0:n], pu[:, 0:n], AF.Tanh)
                    nc.tensor.matmul(pa[0:1, 0:n], uo[c2][:], ut[:, 0:n],
                                     start=(c2 == 0), stop=(c2 == 1))
                asb = utp.tile([1, 512], F32, tag="asb", name="asb")
                nc.vector.tensor_copy(asb[0:1, 0:n], pa[0:1, 0:n])
                nc.sync.dma_start(aout, asb[0:1, 0:n])

            def emit_ctx_tile(nti, hfr, hbr):
                ps_t = pst2.tile([128, 256], BF, tag="psT2", name="psT2")
                nc.tensor.transpose(ps_t[:, 0:128], hfr, identb[:])
                nc.tensor.transpose(ps_t[:, 128:256], hbr, identb[:])
                y = yp.tile([128, 257], BF, tag="y", name="y")
                if nti % 2 == 0:
                    nc.scalar.copy(y[:, 0:256], ps_t[:])
                else:
                    nc.vector.tensor_copy(y[:, 0:256], ps_t[:])
                if nti < 2:
                    nc.vector.memset(y[:, 256:257], 1.0)
                iw = iwp.tile([128, SWIN], BF, tag="iw", name="iw")
                nc.vector.tensor_scalar(iw[:], iota_t[:],
                                        seg_t[:, nti:nti + 1],
                                        e_cm[:, nti:nti + 1],
                                        ALU.is_equal, ALU.mult)
                for k in range(2):
                    nc.tensor.matmul(ctxp[k][:], iw[:, 128 * k:SWIN if k else 128],
                                     y[:], start=(nti == 0), stop=(nti == NTILE))

            for gidx in range(33):
                emit_u_att(gidx)
                if gidx < 32:
                    a4 = anmp.tile([4, 128], F32, tag="a4", name="a4")
                    nc.sync.dma_start(
                        a4[:], att_dram[0:1, 512 * gidx:512 * gidx + 512]
                        .rearrange("a (n p) -> (a n) p", p=128))
                    pe4 = pse.tile([128, 4], F32, tag="pe4", name="pe4")
                    nc.tensor.transpose(pe4[:], a4[:], identf[0:4, 0:4])
                    nc.scalar.activation(e_cm[:, 4 * gidx:4 * gidx + 4], pe4[:],
                                         AF.Exp)
                    for t4 in range(4):
                        nti = 4 * gidx + t4
                        emit_ctx_tile(nti, hfT[:, 128 * nti:128 * nti + 128],
                                      hbT[:, 128 * nti + 127:128 * nti + 255])
                else:
                    att_x = anmp.tile([128, 1], F32, tag="attx", name="attx")
                    nc.sync.dma_start(
                        att_x[:],
                        att_dram[0:1, NQ:NQ + 128].rearrange(
                            "a (n p) -> (a n) p", p=1))
                    nc.scalar.activation(e_cm[:, 128:129], att_x[:], AF.Exp)
                    emit_ctx_tile(NTILE, hfx[:], hbx[:])
            for k in range(2):
                nc.vector.tensor_copy(ctx_sb[k][:], ctxp[k][:])
        for k in range(2):
            nc.sync.dma_start(ctx_out[128 * k:128 * k + (SWIN - 128 if k else 128),
                                      :], ctx_sb[k][:])

    nc.finalize()
    _BUILT["nc"] = nc
    return nc


def _host_prep(inputs):
    x = np.asarray(inputs["sentence"], np.float32)
    doc_mask = np.asarray(inputs["doc_mask"]).astype(np.int64)
    h0g = np.asarray(inputs["h0"], np.float32)
    c0g = np.asarray(inputs["c0"], np.float32)

    perm = np.r_[0:128, 128:256, 384:512, 256:384]  # i,f,o,g order

    def wprep(w):  # [4H, X] -> lhsT [X, 4H] with gate perm, bf16
        return np.ascontiguousarray(w.astype(np.float32).T[:, perm]).astype(BF16)

    wih = {d: wprep(np.asarray(inputs[f"w_ih_{s}"], np.float32))
           for d, s in ((0, "f"), (1, "b"))}
    whh = {d: wprep(np.asarray(inputs[f"w_hh_{s}"], np.float32))
           for d, s in ((0, "f"), (1, "b"))}
    bias = {d: (np.asarray(inputs[f"b_ih_{s}"], np.float32)
                + np.asarray(inputs[f"b_hh_{s}"], np.float32))[perm]
            for d, s in ((0, "f"), (1, "b"))}
    b48 = np.zeros((128, 128), np.float32)
    for d in range(2):
        for k in range(4):
            b48[2 * k + d, :] = bias[d][128 * k:128 * k + 128]
    b48 = b48.astype(BF16)
    i48 = np.zeros((128, 1024), np.float32)
    for r in range(8):
        i48[r, 128 * r:128 * r + 128] = 1.0
    i48 = i48.astype(BF16)

    NSF_, NSB_ = 192, 193
    idxf = (64 + np.arange(NSF_)[:, None] + 128 * np.arange(128)[None, :])
    idxb = (193 - np.arange(NSB_)[:, None] + 129 * np.arange(128)[None, :])

    wom = np.asarray(inputs["w_omega"], np.float32).astype(BF16)
    uo = np.asarray(inputs["u_omega"], np.float32).astype(BF16)
    iota = np.tile(np.arange(SWIN, dtype=np.float32), (128, 1)).astype(BF16)
    identb = np.eye(128, dtype=np.float32).astype(BF16)
    identf = np.eye(128, dtype=np.float32)

    seg_global = np.searchsorted(doc_mask, np.arange(T), side="right")

    in_maps = []
    s_los = []
    xpad = np.zeros((T + 512, D), np.float32)
    xpad[64:64 + T] = x  # global row r ↔ token r - 64
    for c in range(NCORE):
        tc0 = c * PC
        xs = xpad[tc0:tc0 + SH]  # token tc0-64+i at row i
        # step-major gathers: col 128*s + lane
        xsf = np.ascontiguousarray(
            xs[idxf.reshape(-1)].T).astype(BF16)   # [256, 128*NSF]
        xsb = np.ascontiguousarray(
            xs[idxb.reshape(-1)].T).astype(BF16)   # [256, 128*NSB]

        # seeds
        h0f = np.zeros((128, 128), np.float32)
        c0f = np.zeros((128, 128), np.float32)
        h0b = np.zeros((128, 128), np.float32)
        c0b = np.zeros((128, 128), np.float32)
        if c == 0:
            h0f[:, 0] = h0g[0]
            c0f[:, 0] = c0g[0]
        if c == NCORE - 1:
            h0b[:, 126] = h0g[1]
            c0b[:, 126] = c0g[1]

        # segment ids, col-major [128, 129]
        segm = np.full((128, 129), -1.0, np.float32)
        toks_main = tc0 + 64 + np.arange(NQ)
        valid = toks_main < T
        if c == NCORE - 1:
            valid &= (np.arange(NQ) < 16256)  # tail handled by W_tail
        toks_extra = np.full(128, -1, np.int64)
        if c == 0:
            toks_extra[0:64] = np.arange(64)          # W_head: tokens [0,64)
        if c == NCORE - 1:
            toks_extra[64:128] = T - 64 + np.arange(64)  # W_tail
        all_toks = np.concatenate([toks_main[valid],
                                   toks_extra[toks_extra >= 0]])
        s_lo = int(seg_global[all_toks].min()) if all_toks.size else 0
        s_hi = int(seg_global[all_toks].max()) if all_toks.size else 0
        assert s_hi - s_lo < SWIN, f"segment window too wide: {s_hi - s_lo}"
        s_los.append(s_lo)
        sm = np.where(valid, seg_global[np.minimum(toks_main, T - 1)] - s_lo,
                      -1.0).astype(np.float32)
        segm[:, 0:128] = sm.reshape(128, 128).T  # segm[p, n] = seg(q=128n+p)
        se = np.full(128, -1.0, np.float32)
        mask_x = toks_extra >= 0
        se[mask_x] = seg_global[toks_extra[mask_x]] - s_lo
        segm[:, 128] = se

        in_maps.append({
            "xSf": xsf, "xSb": xsb,
            "wih_f": wih[0], "wih_b": wih[1],
            "whh_f": whh[0], "whh_b": whh[1],
            "b48": b48, "i48": i48,
            "h0f": h0f.astype(BF16), "c0f": c0f.astype(BF16),
            "h0b": h0b.astype(BF16), "c0b": c0b.astype(BF16),
            "wom": wom, "uo": uo, "iota": iota,
            "identb": identb, "identf": identf,
            "seg": segm,
        })
    return in_maps, s_los


def kernel(**inputs):
    global LAST_RESULT
    from concourse.bass_utils import run_bass_kernel_spmd

    nc = _build()
    in_maps, s_los = _host_prep(inputs)
    res = run_bass_kernel_spmd(nc, in_maps, core_ids=list(range(NCORE)))
    LAST_RESULT = res

    G = np.zeros((S + SWIN, 257), np.float64)
    for c in range(NCORE):
        ctx = np.asarray(res.results[c]["ctx"], np.float32)
        G[s_los[c]:s_los[c] + SWIN] += ctx
    G = G[:S]
    z = G[:, 256]
    ctx = G[:, :256] / np.where(z == 0, 1.0, z)[:, None]
    w_tag = np.asarray(inputs["w_tag"], np.float32)
    b_tag = np.asarray(inputs["b_tag"], np.float32)
    out = ctx.astype(np.float32) @ w_tag.T + b_tag
    return out.astype(np.float32)



# revision 44
# speedup vs baseline: 1.3873x; 1.0699x over previous
"""Bass/Trainium2 kernel for nn_BiLSTM_Tok_83837761618147.

Strategy (8 NeuronCores, SPMD, full inputs in / full output out):
  - Token dim sharded 8 ways (16384 tokens/core, with halos).
  - BiLSTM parallelized via chunked recurrence with burn-in: each core runs
    128 lanes x (128+64) steps forward and 128 lanes x (129+64) steps
    backward (state forgets exponentially; 64 warmup steps reach fp32
    accuracy; the true h0/c0-seeded lanes cover the sequence ends exactly).
  - Gate pre-activations computed by PE matmuls directly into PSUM
    (bias via a K=4 indicator matmul); w_hh @ h accumulated on top.
  - Attention (tanh/logits/exp) + ragged segment softmax-sum done on
    device via an e-weighted one-hot (token x segment-window) matmul.
  - Host combines per-core partial [segment, 257] sums, normalizes, and
    applies the tiny tag projection.
"""

import numpy as np
import ml_dtypes

BF16 = ml_dtypes.bfloat16

T = 131072
D = 256
H = 128
HID = 256
TAGS = 10
S = 1024
NCORE = 8
PC = T // NCORE          # 16384 tokens per core
B = 64                   # burn-in steps
LF = 128                 # forward lane length (tokens per lane)
LB = 129                 # backward lane length
NL = 128                 # lanes per direction
NSF = B + LF             # 192 forward steps
NSB = B + LB             # 193 backward steps
SH = 16640               # x shard rows [tc0-64, tc0-64+SH)
SWIN = 192               # segment window width per core (max observed 142)
NQ = PC                  # main attention window positions
NTILE = NQ // 128        # 128 main token tiles
HBW = LB * NL - LB + LB + B  # hbT width: 16512
HBT_W = 16512
ATT_W = NQ + 128         # att buffer width (main + extra tile)

_BUILT = {}
LAST_RESULT = None


def _build():
    if "nc" in _BUILT:
        return _BUILT["nc"]
    import contextlib
    from concourse import bacc, mybir
    from concourse.tile import TileContext

    F32 = mybir.dt.float32
    BF = mybir.dt.bfloat16
    AF = mybir.ActivationFunctionType
    ALU = mybir.AluOpType

    nc = bacc.Bacc()

    def din(name, shape, dt):
        return nc.declare_dram_parameter(name, list(shape), dt, isOutput=False)

    xsf_in = din("xSf", [256, 128 * NSF], BF)
    xsb_in = din("xSb", [256, 128 * NSB], BF)
    wih_f_in = din("wih_f", [256, 512], BF)
    wih_b_in = din("wih_b", [256, 512], BF)
    whh_f_in = din("whh_f", [128, 512], BF)
    whh_b_in = din("whh_b", [128, 512], BF)
    b48_in = din("b48", [128, 128], BF)
    i48_in = din("i48", [128, 1024], BF)
    h0f_in = din("h0f", [128, 128], BF)
    c0f_in = din("c0f", [128, 128], BF)
    h0b_in = din("h0b", [128, 128], BF)
    c0b_in = din("c0b", [128, 128], BF)
    wom_in = din("wom", [256, 256], BF)
    uo_in = din("uo", [256, 1], BF)
    iota_in = din("iota", [128, SWIN], BF)
    identb_in = din("identb", [128, 128], BF)
    identf_in = din("identf", [128, 128], F32)
    seg_in = din("seg", [128, 129], F32)
    ctx_out = nc.declare_dram_parameter("ctx", [SWIN, 257], F32, isOutput=True)
    att_dram = nc.dram_tensor("att_stage", [1, ATT_W], F32)

    with TileContext(nc) as tc, contextlib.ExitStack() as ctx:
        pp = ctx.enter_context(tc.tile_pool(name="persist", bufs=1))

        # chunked step-major x staging (CH steps per chunk, double buffered)
        CH = 16
        xf = [[pp.tile([128, 128 * CH], BF, tag=f"xf{b}{kh}", name=f"xf{b}{kh}")
               for kh in range(2)] for b in range(2)]
        xb = [[pp.tile([128, 128 * CH], BF, tag=f"xb{b}{kh}", name=f"xb{b}{kh}")
               for kh in range(2)] for b in range(2)]
        hfT = pp.tile([128, NQ], BF, tag="hfT", name="hfT")
        hbT = pp.tile([128, HBT_W], BF, tag="hbT", name="hbT")
        hf_head = pp.tile([128, 64], BF, tag="hfh", name="hfh")
        hb_head = pp.tile([128, 64], BF, tag="hbh", name="hbh")
        # contiguous recurrent h state (fast writes on the critical chain):
        # hcurF col = 2*lane + (s%2)  (u32-pairable for the hfT scatter)
        # hcurB col = 128*(s%2) + lane
        hcurF = pp.tile([128, 256], BF, tag="hcurF", name="hcurF")
        hcurB = pp.tile([128, 256], BF, tag="hcurB", name="hcurB")
        wih = [[pp.tile([128, 512], BF, tag=f"wih{d}{k}", name=f"wih{d}{k}") for k in range(2)]
               for d in range(2)]
        whh = [pp.tile([128, 512], BF, tag=f"whh{d}", name=f"whh{d}") for d in range(2)]
        b48 = pp.tile([128, 128], BF, tag="b48", name="b48")
        i48 = pp.tile([128, 1024], BF, tag="i48", name="i48")
        h0 = [pp.tile([128, 128], BF, tag=f"h0{d}", name=f"h0{d}") for d in range(2)]
        c0 = [pp.tile([128, 128], BF, tag=f"c0{d}", name=f"c0{d}") for d in range(2)]
        wom = [pp.tile([128, 256], BF, tag=f"wom{k}", name=f"wom{k}") for k in range(2)]
        uo = [pp.tile([128, 1], BF, tag=f"uo{k}", name=f"uo{k}") for k in range(2)]
        iota_t = pp.tile([128, SWIN], BF, tag="iota", name="iota")
        identb = pp.tile([128, 128], BF, tag="identb", name="identb")
        identf = pp.tile([128, 128], F32, tag="identf", name="identf")
        seg_t = pp.tile([128, 129], F32, tag="seg", name="seg")
        CFB = pp.tile([128, 256], BF, tag="CFB", name="CFB")
        e_cm = pp.tile([128, 129], F32, tag="ecm", name="ecm")
        hfx = pp.tile([128, 128], BF, tag="hfx", name="hfx")
        hbx = pp.tile([128, 128], BF, tag="hbx", name="hbx")
        ctx_sb = [pp.tile([128 if k == 0 else SWIN - 128, 257], F32,
                          tag=f"ctxsb{k}", name=f"ctxsb{k}") for k in range(2)]

        # ---- input DMAs ----
        def dma_xchunk(c):
            # stage chunk c (steps [CH*c, CH*c+CH)) of the gathered x
            w = 128 * CH
            if c * CH < NSF:
                wf = min(w, 128 * NSF - c * w)
                for kh in range(2):
                    nc.sync.dma_start(xf[c % 2][kh][:, 0:wf],
                                      xsf_in[128 * kh:128 * kh + 128,
                                             c * w:c * w + wf])
            if c * CH < NSB:
                wb = min(w, 128 * NSB - c * w)
                for kh in range(2):
                    nc.sync.dma_start(xb[c % 2][kh][:, 0:wb],
                                      xsb_in[128 * kh:128 * kh + 128,
                                             c * w:c * w + wb])

        nc.sync.dma_start(b48[:], b48_in[:])
        nc.sync.dma_start(i48[:], i48_in[:])
        for d, t_ in ((0, wih_f_in), (1, wih_b_in)):
            nc.sync.dma_start(wih[d][0][:], t_[0:128, :])
            nc.sync.dma_start(wih[d][1][:], t_[128:256, :])
        nc.sync.dma_start(whh[0][:], whh_f_in[:])
        nc.sync.dma_start(whh[1][:], whh_b_in[:])
        nc.sync.dma_start(h0[0][:], h0f_in[:])
        nc.sync.dma_start(c0[0][:], c0f_in[:])
        nc.sync.dma_start(h0[1][:], h0b_in[:])
        nc.sync.dma_start(c0[1][:], c0b_in[:])
        dma_xchunk(0)
        dma_xchunk(1)
        nc.sync.dma_start(wom[0][:], wom_in[0:128, :])
        nc.sync.dma_start(wom[1][:], wom_in[128:256, :])
        nc.sync.dma_start(uo[0][:], uo_in[0:128, :])
        nc.sync.dma_start(uo[1][:], uo_in[128:256, :])
        nc.sync.dma_start(iota_t[:], iota_in[:])
        nc.sync.dma_start(identb[:], identb_in[:])
        nc.sync.dma_start(identf[:], identf_in[:])
        nc.sync.dma_start(seg_t[:], seg_in[:])

        # init cell state from seeds: CFB = [c0f | c0b]
        nc.vector.tensor_copy(CFB[:, 0:128], c0[0][:])
        nc.vector.tensor_copy(CFB[:, 128:256], c0[1][:])

        def h_src(d, s):
            # h state produced at step s-1 (read at step s)
            if s == 0:
                return h0[d][:]
            p = (s - 1) % 2
            if d == 0:
                return hcurF[:, p:p + 2 * 127 + 1:2]
            return hcurB[:, 128 * p:128 * p + 128]

        def h_dst(d, s):
            p = s % 2
            if d == 0:
                return hcurF[:, p:p + 2 * 127 + 1:2]
            return hcurB[:, 128 * p:128 * p + 128]

        with tc.tile_pool(name="psG", bufs=4, space="PSUM") as psg, \
             tc.tile_pool(name="sig", bufs=3) as sigp, \
             tc.tile_pool(name="tg", bufs=3) as tgp, \
             tc.tile_pool(name="tcn", bufs=3) as tcp, \
             tc.tile_pool(name="tmp1", bufs=3) as t1p, \
             tc.tile_pool(name="tmp2", bufs=3) as t2p:

            G = {}

            def emit_preA(s_):
                # bias + kh0 pre-gate matmuls for step s_ (allocates G tile)
                if s_ >= NSB:
                    return
                g = psg.tile([128, 1024], F32, tag="G", name="G")
                G[s_] = g
                for hb in range(2):
                    nc.tensor.matmul(g[:, 512 * hb:512 * hb + 512], b48[:],
                                     i48[:, 512 * hb:512 * hb + 512],
                                     start=True, stop=False)
                emit_wih(s_, 0)

            def emit_preB(s_):
                if s_ >= NSB:
                    return
                emit_wih(s_, 1)

            def emit_wih(s_, kh):
                g = G[s_]
                o = 128 * (s_ % CH)
                cb = (s_ // CH) % 2
                for d in range(2):
                    if d == 0 and s_ >= NSF:
                        continue
                    xt = xf[cb] if d == 0 else xb[cb]
                    rhs = xt[kh][:, o:o + 128]
                    for j in range(4):
                        nc.tensor.matmul(
                            g[:, 256 * j + 128 * d:256 * j + 128 * d + 128],
                            wih[d][kh][:, 128 * j:128 * j + 128],
                            rhs, start=False, stop=False)

            for s_ in range(3):
                emit_preA(s_)
                emit_preB(s_)
            emit_preA(3)

            for s in range(NSB):
                if s % CH == 0 and s >= CH and (s // CH + 1) * CH < NSB:
                    dma_xchunk(s // CH + 1)
                g = G.pop(s)
                # w_hh matmuls (accumulate into this step's gate region)
                for d in range(2):
                    if d == 0 and s >= NSF:
                        continue
                    hs = h_src(d, s)
                    for j in (0, 1, 3, 2):  # i,f first; g before o
                        nc.tensor.matmul(
                            g[:, 256 * j + 128 * d:256 * j + 128 * d + 128],
                            whh[d][:, 128 * j:128 * j + 128], hs,
                            start=False, stop=True)
                # gates (gate-major layout: cols 256*j + 128*d, j in i,f,o,g)
                # sigmoid split: i,f first (feeds t1/t2, ready after 6 of the
                # 8 whh matmuls), o later (only needed for h at chain end)
                sig = sigp.tile([128, 768], BF, tag="sig", name="sig")
                nc.scalar.activation(sig[:, 0:512], g[:, 0:512], AF.Sigmoid)
                tg = tgp.tile([128, 256], BF, tag="tg", name="tg")
                nc.scalar.activation(tg[:], g[:, 768:1024], AF.Tanh)
                nc.scalar.activation(sig[:, 512:768], g[:, 512:768], AF.Sigmoid)
                # c update (all contiguous [128, 256] = [fwd | bwd])
                t1 = t1p.tile([128, 256], BF, tag="t1", name="t1")
                t2 = t2p.tile([128, 256], BF, tag="t2", name="t2")
                nc.vector.tensor_tensor(t1[:], sig[:, 256:512], CFB[:], ALU.mult)
                nc.vector.tensor_tensor(t2[:], sig[:, 0:256], tg[:], ALU.mult)
                nc.vector.tensor_tensor(CFB[:], t1[:], t2[:], ALU.add)
                tcn = tcp.tile([128, 256], BF, tag="tcn", name="tcn")
                nc.scalar.activation(tcn[:], CFB[:], AF.Tanh)
                # h = sigma_o * tanh(c) into the contiguous recurrent slot
                for d in range(2):
                    if d == 0 and s >= NSF:
                        continue
                    nc.vector.tensor_tensor(h_dst(d, s),
                                            sig[:, 512 + 128 * d:640 + 128 * d],
                                            tcn[:, 128 * d:128 * d + 128],
                                            ALU.mult)
                if s < B:
                    nc.vector.tensor_copy(hf_head[:, s:s + 1],
                                          hcurF[:, s % 2:s % 2 + 1])
                    nc.vector.tensor_copy(hb_head[:, 63 - s:64 - s],
                                          hcurB[:, 128 * (s % 2) + 126:
                                                128 * (s % 2) + 127])
                # off-chain scatters into token-major hfT/hbT
                if s >= B and s < NSF and s % 2 == 1:
                    # fwd pair (s-1, s): u32 packed copy, dst stride 64 u32
                    c0u = (s - 1 - 64) // 2
                    nc.vector.tensor_copy(
                        hfT[:].bitcast(mybir.dt.uint32)[
                            :, c0u:c0u + 64 * 127 + 1:64],
                        hcurF[:].bitcast(mybir.dt.uint32))
                if s >= B:
                    a = 192 - s
                    nc.gpsimd.tensor_copy(hbT[:, a:a + 129 * 127 + 1:129],
                                          hcurB[:, 128 * (s % 2):
                                                128 * (s % 2) + 128])

        # ---------------- attention phase ----------------
        # assemble extra window tiles
        nc.vector.tensor_copy(hfx[:, 0:64], hf_head[:])
        nc.vector.tensor_copy(hfx[:, 64:128], hfT[:, 16256:16320])
        nc.vector.tensor_copy(hbx[:, 0:64], hbT[:, 63:127])
        nc.vector.tensor_copy(hbx[:, 64:128], hb_head[:])

        # merged attention pipeline: per 512-token group, emit u/att matmuls,
        # stage e via a DRAM round-trip + [4,128] transpose, then the 4 ctx
        # tiles (h transposes + e-weighted one-hot accumulation).
        with tc.tile_pool(name="psU", bufs=2, space="PSUM") as psu, \
             tc.tile_pool(name="uT", bufs=3) as utp, \
             tc.tile_pool(name="psA", bufs=1, space="PSUM") as psa, \
             tc.tile_pool(name="anm", bufs=2) as anmp, \
             tc.tile_pool(name="psE", bufs=1, space="PSUM") as pse, \
             tc.tile_pool(name="psT2", bufs=2, space="PSUM") as pst2, \
             tc.tile_pool(name="yp", bufs=2) as yp, \
             tc.tile_pool(name="iw", bufs=2) as iwp, \
             tc.tile_pool(name="psC", bufs=1, space="PSUM") as psc:
            ctxp = [psc.tile([128 if k == 0 else SWIN - 128, 257], F32,
                             tag=f"ctxp{k}", name=f"ctxp{k}") for k in range(2)]

            def emit_u_att(gidx):
                if gidx < 32:
                    n = 512
                    hfr = hfT[:, 512 * gidx:512 * gidx + 512]
                    hbr = hbT[:, 512 * gidx + 127:512 * gidx + 127 + 512]
                    aout = att_dram[0:1, 512 * gidx:512 * gidx + 512]
                else:
                    n = 128
                    hfr = hfx[:]
                    hbr = hbx[:]
                    aout = att_dram[0:1, NQ:NQ + 128]
                pa = psa.tile([1, 512], F32, tag="psA", name="psA")
                for c2 in range(2):
                    pu = psu.tile([128, 512], F32, tag="psU", name="psU")
                    nc.tensor.matmul(pu[:, 0:n], wom[0][:, 128 * c2:128 * c2 + 128],
                                     hfr, start=True, stop=False)
                    nc.tensor.matmul(pu[:, 0:n], wom[1][:, 128 * c2:128 * c2 + 128],
                                     hbr, start=False, stop=True)
                    ut = utp.tile([128, 512], BF, tag="uT", name="uT")
                    nc.scalar.activation(ut[:, 0:n], pu[:, 0:n], AF.Tanh)
                    nc.tensor.matmul(pa[0:1, 0:n], uo[c2][:], ut[:, 0:n],
                                     start=(c2 == 0), stop=(c2 == 1))
                asb = utp.tile([1, 512], F32, tag="asb", name="asb")
                nc.vector.tensor_copy(asb[0:1, 0:n], pa[0:1, 0:n])
                nc.sync.dma_start(aout, asb[0:1, 0:n])

            def emit_ctx_tile(nti, hfr, hbr):
                ps_t = pst2.tile([128, 256], BF, tag="psT2", name="psT2")
                nc.tensor.transpose(ps_t[:, 0:128], hfr, identb[:])
                nc.tensor.transpose(ps_t[:, 128:256], hbr, identb[:])
                y = yp.tile([128, 257], BF, tag="y", name="y")
                if nti % 2 == 0:
                    nc.scalar.copy(y[:, 0:256], ps_t[:])
                else:
                    nc.vector.tensor_copy(y[:, 0:256], ps_t[:])
                if nti < 2:
                    nc.vector.memset(y[:, 256:257], 1.0)
                iw = iwp.tile([128, SWIN], BF, tag="iw", name="iw")
                nc.vector.tensor_scalar(iw[:], iota_t[:],
                                        seg_t[:, nti:nti + 1],
                                        e_cm[:, nti:nti + 1],
                                        ALU.is_equal, ALU.mult)
                for k in range(2):
                    nc.tensor.matmul(ctxp[k][:], iw[:, 128 * k:SWIN if k else 128],
                                     y[:], start=(nti == 0), stop=(nti == NTILE))

            for gidx in range(33):
                emit_u_att(gidx)
                if gidx < 32:
                    a4 = anmp.tile([4, 128], F32, tag="a4", name="a4")
                    nc.sync.dma_start(
                        a4[:], att_dram[0:1, 512 * gidx:512 * gidx + 512]
                        .rearrange("a (n p) -> (a n) p", p=128))
                    pe4 = pse.tile([128, 4], F32, tag="pe4", name="pe4")
                    nc.tensor.transpose(pe4[:], a4[:], identf[0:4, 0:4])
                    nc.scalar.activation(e_cm[:, 4 * gidx:4 * gidx + 4], pe4[:],
                                         AF.Exp)
                    for t4 in range(4):
                        nti = 4 * gidx + t4
                        emit_ctx_tile(nti, hfT[:, 128 * nti:128 * nti + 128],
                                      hbT[:, 128 * nti + 127:128 * nti + 255])
                else:
                    att_x = anmp.tile([128, 1], F32, tag="attx", name="attx")
                    nc.sync.dma_start(
                        att_x[:],
                        att_dram[0:1, NQ:NQ + 128].rearrange(
                            "a (n p) -> (a n) p", p=1))
                    nc.scalar.activation(e_cm[:, 128:129], att_x[:], AF.Exp)
                    emit_ctx_tile(NTILE, hfx[:], hbx[:])
            for k in range(2):
                nc.vector.tensor_copy(ctx_sb[k][:], ctxp[k][:])
        for k in range(2):
            nc.sync.dma_start(ctx_out[128 * k:128 * k + (SWIN - 128 if k else 128),
                                      :], ctx_sb[k][:])

    nc.finalize()
    _BUILT["nc"] = nc
    return nc


def _host_prep(inputs):
    x = np.asarray(inputs["sentence"], np.float32)
    doc_mask = np.asarray(inputs["doc_mask"]).astype(np.int64)
    h0g = np.asarray(inputs["h0"], np.float32)
    c0g = np.asarray(inputs["c0"], np.float32)

    perm = np.r_[0:128, 128:256, 384:512, 256:384]  # i,f,o,g order

    def wprep(w):  # [4H, X] -> lhsT [X, 4H] with gate perm, bf16
        return np.ascontiguousarray(w.astype(np.float32).T[:, perm]).astype(BF16)

    wih = {d: wprep(np.asarray(inputs[f"w_ih_{s}"], np.float32))
           for d, s in ((0, "f"), (1, "b"))}
    whh = {d: wprep(np.asarray(inputs[f"w_hh_{s}"], np.float32))
           for d, s in ((0, "f"), (1, "b"))}
    bias = {d: (np.asarray(inputs[f"b_ih_{s}"], np.float32)
                + np.asarray(inputs[f"b_hh_{s}"], np.float32))[perm]
            for d, s in ((0, "f"), (1, "b"))}
    b48 = np.zeros((128, 128), np.float32)
    for d in range(2):
        for k in range(4):
            b48[2 * k + d, :] = bias[d][128 * k:128 * k + 128]
    b48 = b48.astype(BF16)
    i48 = np.zeros((128, 1024), np.float32)
    for r in range(8):
        i48[r, 128 * r:128 * r + 128] = 1.0
    i48 = i48.astype(BF16)

    NSF_, NSB_ = 192, 193
    idxf = (64 + np.arange(NSF_)[:, None] + 128 * np.arange(128)[None, :])
    idxb = (193 - np.arange(NSB_)[:, None] + 129 * np.arange(128)[None, :])

    wom = np.asarray(inputs["w_omega"], np.float32).astype(BF16)
    uo = np.asarray(inputs["u_omega"], np.float32).astype(BF16)
    iota = np.tile(np.arange(SWIN, dtype=np.float32), (128, 1)).astype(BF16)
    identb = np.eye(128, dtype=np.float32).astype(BF16)
    identf = np.eye(128, dtype=np.float32)

    seg_global = np.searchsorted(doc_mask, np.arange(T), side="right")

    in_maps = []
    s_los = []
    xpad = np.zeros((T + 512, D), np.float32)
    xpad[64:64 + T] = x  # global row r ↔ token r - 64
    for c in range(NCORE):
        tc0 = c * PC
        xs = xpad[tc0:tc0 + SH]  # token tc0-64+i at row i
        # step-major gathers: col 128*s + lane
        xsf = np.ascontiguousarray(
            xs[idxf.reshape(-1)].T).astype(BF16)   # [256, 128*NSF]
        xsb = np.ascontiguousarray(
            xs[idxb.reshape(-1)].T).astype(BF16)   # [256, 128*NSB]

        # seeds
        h0f = np.zeros((128, 128), np.float32)
        c0f = np.zeros((128, 128), np.float32)
        h0b = np.zeros((128, 128), np.float32)
        c0b = np.zeros((128, 128), np.float32)
        if c == 0:
            h0f[:, 0] = h0g[0]
            c0f[:, 0] = c0g[0]
        if c == NCORE - 1:
            h0b[:, 126] = h0g[1]
            c0b[:, 126] = c0g[1]

        # segment ids, col-major [128, 129]
        segm = np.full((128, 129), -1.0, np.float32)
        toks_main = tc0 + 64 + np.arange(NQ)
        valid = toks_main < T
        if c == NCORE - 1:
            valid &= (np.arange(NQ) < 16256)  # tail handled by W_tail
        toks_extra = np.full(128, -1, np.int64)
        if c == 0:
            toks_extra[0:64] = np.arange(64)          # W_head: tokens [0,64)
        if c == NCORE - 1:
            toks_extra[64:128] = T - 64 + np.arange(64)  # W_tail
        all_toks = np.concatenate([toks_main[valid],
                                   toks_extra[toks_extra >= 0]])
        s_lo = int(seg_global[all_toks].min()) if all_toks.size else 0
        s_hi = int(seg_global[all_toks].max()) if all_toks.size else 0
        assert s_hi - s_lo < SWIN, f"segment window too wide: {s_hi - s_lo}"
        s_los.append(s_lo)
        sm = np.where(valid, seg_global[np.minimum(toks_main, T - 1)] - s_lo,
                      -1.0).astype(np.float32)
        segm[:, 0:128] = sm.reshape(128, 128).T  # segm[p, n] = seg(q=128n+p)
        se = np.full(128, -1.0, np.float32)
        mask_x = toks_extra >= 0
        se[mask_x] = seg_global[toks_extra[mask_x]] - s_lo
        segm[:, 128] = se

        in_maps.append({
            "xSf": xsf, "xSb": xsb,
            "wih_f": wih[0], "wih_b": wih[1],
            "whh_f": whh[0], "whh_b": whh[1],
            "b48": b48, "i48": i48,
            "h0f": h0f.astype(BF16), "c0f": c0f.astype(BF16),
            "h0b": h0b.astype(BF16), "c0b": c0b.astype(BF16),
            "wom": wom, "uo": uo, "iota": iota,
            "identb": identb, "identf": identf,
            "seg": segm,
        })
    return in_maps, s_los


def kernel(**inputs):
    global LAST_RESULT
    from concourse.bass_utils import run_bass_kernel_spmd

    nc = _build()
    in_maps, s_los = _host_prep(inputs)
    res = run_bass_kernel_spmd(nc, in_maps, core_ids=list(range(NCORE)))
    LAST_RESULT = res

    G = np.zeros((S + SWIN, 257), np.float64)
    for c in range(NCORE):
        ctx = np.asarray(res.results[c]["ctx"], np.float32)
        G[s_los[c]:s_los[c] + SWIN] += ctx
    G = G[:S]
    z = G[:, 256]
    ctx = G[:, :256] / np.where(z == 0, 1.0, z)[:, None]
    w_tag = np.asarray(inputs["w_tag"], np.float32)
    b_tag = np.asarray(inputs["b_tag"], np.float32)
    out = ctx.astype(np.float32) @ w_tag.T + b_tag
    return out.astype(np.float32)



# revision 47
# speedup vs baseline: 1.4275x; 1.0289x over previous
"""Bass/Trainium2 kernel for nn_BiLSTM_Tok_83837761618147.

Strategy (8 NeuronCores, SPMD, full inputs in / full output out):
  - Token dim sharded 8 ways (16384 tokens/core, with halos).
  - BiLSTM parallelized via chunked recurrence with burn-in: each core runs
    128 lanes x (128+64) steps forward and 128 lanes x (129+64) steps
    backward (state forgets exponentially; 64 warmup steps reach fp32
    accuracy; the true h0/c0-seeded lanes cover the sequence ends exactly).
  - Gate pre-activations computed by PE matmuls directly into PSUM
    (bias via a K=4 indicator matmul); w_hh @ h accumulated on top.
  - Attention (tanh/logits/exp) + ragged segment softmax-sum done on
    device via an e-weighted one-hot (token x segment-window) matmul.
  - Host combines per-core partial [segment, 257] sums, normalizes, and
    applies the tiny tag projection.
"""

import numpy as np
import ml_dtypes

BF16 = ml_dtypes.bfloat16

T = 131072
D = 256
H = 128
HID = 256
TAGS = 10
S = 1024
NCORE = 8
PC = T // NCORE          # 16384 tokens per core
B = 64                   # burn-in steps
LF = 128                 # forward lane length (tokens per lane)
LB = 129                 # backward lane length
NL = 128                 # lanes per direction
NSF = B + LF             # 192 forward steps
NSB = B + LB             # 193 backward steps
SH = 16640               # x shard rows [tc0-64, tc0-64+SH)
SWIN = 192               # segment window width per core (max observed 142)
NQ = PC                  # main attention window positions
NTILE = NQ // 128        # 128 main token tiles
HBW = LB * NL - LB + LB + B  # hbT width: 16512
HBT_W = 16512
ATT_W = NQ + 128         # att buffer width (main + extra tile)

_BUILT = {}
LAST_RESULT = None


def _build():
    if "nc" in _BUILT:
        return _BUILT["nc"]
    import contextlib
    from concourse import bacc, mybir
    from concourse.tile import TileContext

    F32 = mybir.dt.float32
    BF = mybir.dt.bfloat16
    AF = mybir.ActivationFunctionType
    ALU = mybir.AluOpType

    nc = bacc.Bacc()

    def din(name, shape, dt):
        return nc.declare_dram_parameter(name, list(shape), dt, isOutput=False)

    xsf_in = din("xSf", [256, 128 * NSF], BF)
    xsb_in = din("xSb", [256, 128 * NSB], BF)
    wih_f_in = din("wih_f", [256, 512], BF)
    wih_b_in = din("wih_b", [256, 512], BF)
    whh_f_in = din("whh_f", [128, 512], BF)
    whh_b_in = din("whh_b", [128, 512], BF)
    b48_in = din("b48", [128, 128], BF)
    i48_in = din("i48", [128, 1024], BF)
    h0f_in = din("h0f", [128, 128], BF)
    c0f_in = din("c0f", [128, 128], BF)
    h0b_in = din("h0b", [128, 128], BF)
    c0b_in = din("c0b", [128, 128], BF)
    wom_in = din("wom", [256, 256], BF)
    uo_in = din("uo", [256, 1], BF)
    iota_in = din("iota", [128, SWIN], BF)
    identb_in = din("identb", [128, 128], BF)
    identf_in = din("identf", [128, 128], F32)
    seg_in = din("seg", [128, 129], F32)
    ctx_out = nc.declare_dram_parameter("ctx", [SWIN, 257], F32, isOutput=True)
    att_dram = nc.dram_tensor("att_stage", [1, ATT_W], F32)

    with TileContext(nc) as tc, contextlib.ExitStack() as ctx:
        pp = ctx.enter_context(tc.tile_pool(name="persist", bufs=1))

        # chunked step-major x staging (CH steps per chunk, double buffered)
        CH = 16
        xf = [[pp.tile([128, 128 * CH], BF, tag=f"xf{b}{kh}", name=f"xf{b}{kh}")
               for kh in range(2)] for b in range(2)]
        xb = [[pp.tile([128, 128 * CH], BF, tag=f"xb{b}{kh}", name=f"xb{b}{kh}")
               for kh in range(2)] for b in range(2)]
        hfT = pp.tile([128, NQ], BF, tag="hfT", name="hfT")
        hbT = pp.tile([128, HBT_W], BF, tag="hbT", name="hbT")
        hf_head = pp.tile([128, 64], BF, tag="hfh", name="hfh")
        hb_head = pp.tile([128, 64], BF, tag="hbh", name="hbh")
        # contiguous recurrent h state (fast writes on the critical chain):
        # hcurF col = 2*lane + (s%2)  (u32-pairable for the hfT scatter)
        # hcurB col = 128*(s%2) + lane
        hcurF = pp.tile([128, 256], BF, tag="hcurF", name="hcurF")
        hcurB = pp.tile([128, 256], BF, tag="hcurB", name="hcurB")
        wih = [[pp.tile([128, 512], BF, tag=f"wih{d}{k}", name=f"wih{d}{k}") for k in range(2)]
               for d in range(2)]
        whh = [pp.tile([128, 512], BF, tag=f"whh{d}", name=f"whh{d}") for d in range(2)]
        b48 = pp.tile([128, 128], BF, tag="b48", name="b48")
        i48 = pp.tile([128, 1024], BF, tag="i48", name="i48")
        h0 = [pp.tile([128, 128], BF, tag=f"h0{d}", name=f"h0{d}") for d in range(2)]
        c0 = [pp.tile([128, 128], BF, tag=f"c0{d}", name=f"c0{d}") for d in range(2)]
        wom = [pp.tile([128, 256], BF, tag=f"wom{k}", name=f"wom{k}") for k in range(2)]
        uo = [pp.tile([128, 1], BF, tag=f"uo{k}", name=f"uo{k}") for k in range(2)]
        iota_t = pp.tile([128, SWIN], BF, tag="iota", name="iota")
        identb = pp.tile([128, 128], BF, tag="identb", name="identb")
        identf = pp.tile([128, 128], F32, tag="identf", name="identf")
        seg_t = pp.tile([128, 129], F32, tag="seg", name="seg")
        CFB = pp.tile([128, 256], BF, tag="CFB", name="CFB")
        e_cm = pp.tile([128, 129], F32, tag="ecm", name="ecm")
        hfx = pp.tile([128, 128], BF, tag="hfx", name="hfx")
        hbx = pp.tile([128, 128], BF, tag="hbx", name="hbx")
        ctx_sb = [pp.tile([128 if k == 0 else SWIN - 128, 257], F32,
                          tag=f"ctxsb{k}", name=f"ctxsb{k}") for k in range(2)]

        # ---- input DMAs ----
        def dma_xchunk(c):
            # stage chunk c (steps [CH*c, CH*c+CH)) of the gathered x
            w = 128 * CH
            if c * CH < NSF:
                wf = min(w, 128 * NSF - c * w)
                for kh in range(2):
                    nc.sync.dma_start(xf[c % 2][kh][:, 0:wf],
                                      xsf_in[128 * kh:128 * kh + 128,
                                             c * w:c * w + wf])
            if c * CH < NSB:
                wb = min(w, 128 * NSB - c * w)
                for kh in range(2):
                    nc.sync.dma_start(xb[c % 2][kh][:, 0:wb],
                                      xsb_in[128 * kh:128 * kh + 128,
                                             c * w:c * w + wb])

        nc.sync.dma_start(b48[:], b48_in[:])
        nc.sync.dma_start(i48[:], i48_in[:])
        for d, t_ in ((0, wih_f_in), (1, wih_b_in)):
            nc.sync.dma_start(wih[d][0][:], t_[0:128, :])
            nc.sync.dma_start(wih[d][1][:], t_[128:256, :])
        nc.sync.dma_start(whh[0][:], whh_f_in[:])
        nc.sync.dma_start(whh[1][:], whh_b_in[:])
        nc.sync.dma_start(h0[0][:], h0f_in[:])
        nc.sync.dma_start(c0[0][:], c0f_in[:])
        nc.sync.dma_start(h0[1][:], h0b_in[:])
        nc.sync.dma_start(c0[1][:], c0b_in[:])
        dma_xchunk(0)
        dma_xchunk(1)
        nc.sync.dma_start(wom[0][:], wom_in[0:128, :])
        nc.sync.dma_start(wom[1][:], wom_in[128:256, :])
        nc.sync.dma_start(uo[0][:], uo_in[0:128, :])
        nc.sync.dma_start(uo[1][:], uo_in[128:256, :])
        nc.sync.dma_start(iota_t[:], iota_in[:])
        nc.sync.dma_start(identb[:], identb_in[:])
        nc.sync.dma_start(identf[:], identf_in[:])
        nc.sync.dma_start(seg_t[:], seg_in[:])

        # init cell state from seeds: CFB = [c0f | c0b]
        nc.vector.tensor_copy(CFB[:, 0:128], c0[0][:])
        nc.vector.tensor_copy(CFB[:, 128:256], c0[1][:])

        def h_src(d, s):
            # h state produced at step s-1 (read at step s)
            if s == 0:
                return h0[d][:]
            p = (s - 1) % 2
            if d == 0:
                return hcurF[:, p:p + 2 * 127 + 1:2]
            return hcurB[:, 128 * p:128 * p + 128]

        def h_dst(d, s):
            p = s % 2
            if d == 0:
                return hcurF[:, p:p + 2 * 127 + 1:2]
            return hcurB[:, 128 * p:128 * p + 128]

        with tc.tile_pool(name="psG", bufs=4, space="PSUM") as psg, \
             tc.tile_pool(name="sig", bufs=3) as sigp, \
             tc.tile_pool(name="tg", bufs=3) as tgp, \
             tc.tile_pool(name="tcn", bufs=3) as tcp, \
             tc.tile_pool(name="tmp1", bufs=3) as t1p, \
             tc.tile_pool(name="tmp2", bufs=3) as t2p:

            G = {}

            def emit_preA(s_):
                # bias + kh0 pre-gate matmuls for step s_ (allocates G tile)
                if s_ >= NSB:
                    return
                g = psg.tile([128, 1024], F32, tag="G", name="G")
                G[s_] = g
                for hb in range(2):
                    nc.tensor.matmul(g[:, 512 * hb:512 * hb + 512], b48[:],
                                     i48[:, 512 * hb:512 * hb + 512],
                                     start=True, stop=False)
                emit_wih(s_, 0)

            def emit_preB(s_):
                if s_ >= NSB:
                    return
                emit_wih(s_, 1)

            def emit_wih(s_, kh):
                g = G[s_]
                o = 128 * (s_ % CH)
                cb = (s_ // CH) % 2
                for d in range(2):
                    if d == 0 and s_ >= NSF:
                        continue
                    xt = xf[cb] if d == 0 else xb[cb]
                    rhs = xt[kh][:, o:o + 128]
                    for j in range(4):
                        nc.tensor.matmul(
                            g[:, 256 * j + 128 * d:256 * j + 128 * d + 128],
                            wih[d][kh][:, 128 * j:128 * j + 128],
                            rhs, start=False, stop=False)

            for s_ in range(3):
                emit_preA(s_)
                emit_preB(s_)
            emit_preA(3)

            for s in range(NSB):
                if s % CH == 0 and s >= CH and (s // CH + 1) * CH < NSB:
                    dma_xchunk(s // CH + 1)
                g = G.pop(s)
                # w_hh matmuls (accumulate into this step's gate region)
                for d in range(2):
                    if d == 0 and s >= NSF:
                        continue
                    hs = h_src(d, s)
                    for j in (0, 1, 3, 2):  # i,f first; g before o
                        nc.tensor.matmul(
                            g[:, 256 * j + 128 * d:256 * j + 128 * d + 128],
                            whh[d][:, 128 * j:128 * j + 128], hs,
                            start=False, stop=True)
                # gates (gate-major layout: cols 256*j + 128*d, j in i,f,o,g)
                # sigmoid split: i,f first (feeds t1/t2, ready after 6 of the
                # 8 whh matmuls), o later (only needed for h at chain end)
                sig = sigp.tile([128, 768], BF, tag="sig", name="sig")
                nc.scalar.activation(sig[:, 0:512], g[:, 0:512], AF.Sigmoid)
                tg = tgp.tile([128, 256], BF, tag="tg", name="tg")
                nc.scalar.activation(tg[:], g[:, 768:1024], AF.Tanh)
                nc.scalar.activation(sig[:, 512:768], g[:, 512:768], AF.Sigmoid)
                # c update; t1/t2 merged, then a per-direction tail so the
                # fwd half (add -> tanh_c -> h) completes early and unblocks
                # whh_F(s+1). None of these read G, so no PSUM-WAR impact.
                t1 = t1p.tile([128, 256], BF, tag="t1", name="t1")
                t2 = t2p.tile([128, 256], BF, tag="t2", name="t2")
                nc.vector.tensor_tensor(t1[:], sig[:, 256:512], CFB[:], ALU.mult)
                nc.vector.tensor_tensor(t2[:], sig[:, 0:256], tg[:], ALU.mult)
                tcn = tcp.tile([128, 256], BF, tag="tcn", name="tcn")
                for d in range(2):
                    nc.vector.tensor_tensor(CFB[:, 128 * d:128 * d + 128],
                                            t1[:, 128 * d:128 * d + 128],
                                            t2[:, 128 * d:128 * d + 128],
                                            ALU.add)
                for d in range(2):
                    nc.scalar.activation(tcn[:, 128 * d:128 * d + 128],
                                         CFB[:, 128 * d:128 * d + 128], AF.Tanh)
                for d in range(2):
                    if d == 0 and s >= NSF:
                        continue
                    nc.vector.tensor_tensor(h_dst(d, s),
                                            sig[:, 512 + 128 * d:640 + 128 * d],
                                            tcn[:, 128 * d:128 * d + 128],
                                            ALU.mult)
                if s < B:
                    nc.vector.tensor_copy(hf_head[:, s:s + 1],
                                          hcurF[:, s % 2:s % 2 + 1])
                    nc.vector.tensor_copy(hb_head[:, 63 - s:64 - s],
                                          hcurB[:, 128 * (s % 2) + 126:
                                                128 * (s % 2) + 127])
                # off-chain scatters into token-major hfT/hbT
                if s >= B and s < NSF and s % 2 == 1:
                    # fwd pair (s-1, s): u32 packed copy, dst stride 64 u32
                    c0u = (s - 1 - 64) // 2
                    nc.vector.tensor_copy(
                        hfT[:].bitcast(mybir.dt.uint32)[
                            :, c0u:c0u + 64 * 127 + 1:64],
                        hcurF[:].bitcast(mybir.dt.uint32))
                if s >= B:
                    a = 192 - s
                    nc.gpsimd.tensor_copy(hbT[:, a:a + 129 * 127 + 1:129],
                                          hcurB[:, 128 * (s % 2):
                                                128 * (s % 2) + 128])
                # pre-gate work for future steps, split so the half gated on
                # this step's PSUM-buffer release (preA s+4) queues after the
                # ungated half; whh(s+1) lands behind them with minimal skew
                emit_preB(s + 3)
                emit_preA(s + 4)

        # ---------------- attention phase ----------------
        # assemble extra window tiles
        nc.vector.tensor_copy(hfx[:, 0:64], hf_head[:])
        nc.vector.tensor_copy(hfx[:, 64:128], hfT[:, 16256:16320])
        nc.vector.tensor_copy(hbx[:, 0:64], hbT[:, 63:127])
        nc.vector.tensor_copy(hbx[:, 64:128], hb_head[:])

        # merged attention pipeline: per 512-token group, emit u/att matmuls,
        # stage e via a DRAM round-trip + [4,128] transpose, then the 4 ctx
        # tiles (h transposes + e-weighted one-hot accumulation).
        with tc.tile_pool(name="psU", bufs=2, space="PSUM") as psu, \
             tc.tile_pool(name="uT", bufs=3) as utp, \
             tc.tile_pool(name="psA", bufs=1, space="PSUM") as psa, \
             tc.tile_pool(name="anm", bufs=2) as anmp, \
             tc.tile_pool(name="psE", bufs=1, space="PSUM") as pse, \
             tc.tile_pool(name="psT2", bufs=2, space="PSUM") as pst2, \
             tc.tile_pool(name="yp", bufs=2) as yp, \
             tc.tile_pool(name="iw", bufs=2) as iwp, \
             tc.tile_pool(name="psC", bufs=1, space="PSUM") as psc:
            ctxp = [psc.tile([128 if k == 0 else SWIN - 128, 257], F32,
                             tag=f"ctxp{k}", name=f"ctxp{k}") for k in range(2)]

            def emit_u_att(gidx):
                if gidx < 32:
                    n = 512
                    hfr = hfT[:, 512 * gidx:512 * gidx + 512]
                    hbr = hbT[:, 512 * gidx + 127:512 * gidx + 127 + 512]
                    aout = att_dram[0:1, 512 * gidx:512 * gidx + 512]
                else:
                    n = 128
                    hfr = hfx[:]
                    hbr = hbx[:]
                    aout = att_dram[0:1, NQ:NQ + 128]
                pa = psa.tile([1, 512], F32, tag="psA", name="psA")
                for c2 in range(2):
                    pu = psu.tile([128, 512], F32, tag="psU", name="psU")
                    nc.tensor.matmul(pu[:, 0:n], wom[0][:, 128 * c2:128 * c2 + 128],
                                     hfr, start=True, stop=False)
                    nc.tensor.matmul(pu[:, 0:n], wom[1][:, 128 * c2:128 * c2 + 128],
                                     hbr, start=False, stop=True)
                    ut = utp.tile([128, 512], BF, tag="uT", name="uT")
                    nc.scalar.activation(ut[:, 0:n], pu[:, 0:n], AF.Tanh)
                    nc.tensor.matmul(pa[0:1, 0:n], uo[c2][:], ut[:, 0:n],
                                     start=(c2 == 0), stop=(c2 == 1))
                asb = utp.tile([1, 512], F32, tag="asb", name="asb")
                nc.vector.tensor_copy(asb[0:1, 0:n], pa[0:1, 0:n])
                nc.sync.dma_start(aout, asb[0:1, 0:n])

            def emit_ctx_tile(nti, hfr, hbr):
                ps_t = pst2.tile([128, 256], BF, tag="psT2", name="psT2")
                nc.tensor.transpose(ps_t[:, 0:128], hfr, identb[:])
                nc.tensor.transpose(ps_t[:, 128:256], hbr, identb[:])
                y = yp.tile([128, 257], BF, tag="y", name="y")
                if nti % 2 == 0:
                    nc.scalar.copy(y[:, 0:256], ps_t[:])
                else:
                    nc.vector.tensor_copy(y[:, 0:256], ps_t[:])
                if nti < 2:
                    nc.vector.memset(y[:, 256:257], 1.0)
                iw = iwp.tile([128, SWIN], BF, tag="iw", name="iw")
                nc.vector.tensor_scalar(iw[:], iota_t[:],
                                        seg_t[:, nti:nti + 1],
                                        e_cm[:, nti:nti + 1],
                                        ALU.is_equal, ALU.mult)
                for k in range(2):
                    nc.tensor.matmul(ctxp[k][:], iw[:, 128 * k:SWIN if k else 128],
                                     y[:], start=(nti == 0), stop=(nti == NTILE))

            for gidx in range(33):
                emit_u_att(gidx)
                if gidx < 32:
                    a4 = anmp.tile([4, 128], F32, tag="a4", name="a4")
                    nc.sync.dma_start(
                        a4[:], att_dram[0:1, 512 * gidx:512 * gidx + 512]
                        .rearrange("a (n p) -> (a n) p", p=128))
                    pe4 = pse.tile([128, 4], F32, tag="pe4", name="pe4")
                    nc.tensor.transpose(pe4[:], a4[:], identf[0:4, 0:4])
                    nc.scalar.activation(e_cm[:, 4 * gidx:4 * gidx + 4], pe4[:],
                                         AF.Exp)
                    for t4 in range(4):
                        nti = 4 * gidx + t4
                        emit_ctx_tile(nti, hfT[:, 128 * nti:128 * nti + 128],
                                      hbT[:, 128 * nti + 127:128 * nti + 255])
                else:
                    att_x = anmp.tile([128, 1], F32, tag="attx", name="attx")
                    nc.sync.dma_start(
                        att_x[:],
                        att_dram[0:1, NQ:NQ + 128].rearrange(
                            "a (n p) -> (a n) p", p=1))
                    nc.scalar.activation(e_cm[:, 128:129], att_x[:], AF.Exp)
                    emit_ctx_tile(NTILE, hfx[:], hbx[:])
            for k in range(2):
                nc.vector.tensor_copy(ctx_sb[k][:], ctxp[k][:])
        for k in range(2):
            nc.sync.dma_start(ctx_out[128 * k:128 * k + (SWIN - 128 if k else 128),
                                      :], ctx_sb[k][:])

    nc.finalize()
    _BUILT["nc"] = nc
    return nc


def _host_prep(inputs):
    x = np.asarray(inputs["sentence"], np.float32)
    doc_mask = np.asarray(inputs["doc_mask"]).astype(np.int64)
    h0g = np.asarray(inputs["h0"], np.float32)
    c0g = np.asarray(inputs["c0"], np.float32)

    perm = np.r_[0:128, 128:256, 384:512, 256:384]  # i,f,o,g order

    def wprep(w):  # [4H, X] -> lhsT [X, 4H] with gate perm, bf16
        return np.ascontiguousarray(w.astype(np.float32).T[:, perm]).astype(BF16)

    wih = {d: wprep(np.asarray(inputs[f"w_ih_{s}"], np.float32))
           for d, s in ((0, "f"), (1, "b"))}
    whh = {d: wprep(np.asarray(inputs[f"w_hh_{s}"], np.float32))
           for d, s in ((0, "f"), (1, "b"))}
    bias = {d: (np.asarray(inputs[f"b_ih_{s}"], np.float32)
                + np.asarray(inputs[f"b_hh_{s}"], np.float32))[perm]
            for d, s in ((0, "f"), (1, "b"))}
    b48 = np.zeros((128, 128), np.float32)
    for d in range(2):
        for k in range(4):
            b48[2 * k + d, :] = bias[d][128 * k:128 * k + 128]
    b48 = b48.astype(BF16)
    i48 = np.zeros((128, 1024), np.float32)
    for r in range(8):
        i48[r, 128 * r:128 * r + 128] = 1.0
    i48 = i48.astype(BF16)

    NSF_, NSB_ = 192, 193
    idxf = (64 + np.arange(NSF_)[:, None] + 128 * np.arange(128)[None, :])
    idxb = (193 - np.arange(NSB_)[:, None] + 129 * np.arange(128)[None, :])

    wom = np.asarray(inputs["w_omega"], np.float32).astype(BF16)
    uo = np.asarray(inputs["u_omega"], np.float32).astype(BF16)
    iota = np.tile(np.arange(SWIN, dtype=np.float32), (128, 1)).astype(BF16)
    identb = np.eye(128, dtype=np.float32).astype(BF16)
    identf = np.eye(128, dtype=np.float32)

    seg_global = np.searchsorted(doc_mask, np.arange(T), side="right")

    in_maps = []
    s_los = []
    xpad = np.zeros((T + 512, D), np.float32)
    xpad[64:64 + T] = x  # global row r ↔ token r - 64
    for c in range(NCORE):
        tc0 = c * PC
        xs = xpad[tc0:tc0 + SH]  # token tc0-64+i at row i
        # step-major gathers: col 128*s + lane
        xsf = np.ascontiguousarray(
            xs[idxf.reshape(-1)].T).astype(BF16)   # [256, 128*NSF]
        xsb = np.ascontiguousarray(
            xs[idxb.reshape(-1)].T).astype(BF16)   # [256, 128*NSB]

        # seeds
        h0f = np.zeros((128, 128), np.float32)
        c0f = np.zeros((128, 128), np.float32)
        h0b = np.zeros((128, 128), np.float32)
        c0b = np.zeros((128, 128), np.float32)
        if c == 0:
            h0f[:, 0] = h0g[0]
            c0f[:, 0] = c0g[0]
        if c == NCORE - 1:
            h0b[:, 126] = h0g[1]
            c0b[:, 126] = c0g[1]

        # segment ids, col-major [128, 129]
        segm = np.full((128, 129), -1.0, np.float32)
        toks_main = tc0 + 64 + np.arange(NQ)
        valid = toks_main < T
        if c == NCORE - 1:
            valid &= (np.arange(NQ) < 16256)  # tail handled by W_tail
        toks_extra = np.full(128, -1, np.int64)
        if c == 0:
            toks_extra[0:64] = np.arange(64)          # W_head: tokens [0,64)
        if c == NCORE - 1:
            toks_extra[64:128] = T - 64 + np.arange(64)  # W_tail
        all_toks = np.concatenate([toks_main[valid],
                                   toks_extra[toks_extra >= 0]])
        s_lo = int(seg_global[all_toks].min()) if all_toks.size else 0
        s_hi = int(seg_global[all_toks].max()) if all_toks.size else 0
        assert s_hi - s_lo < SWIN, f"segment window too wide: {s_hi - s_lo}"
        s_los.append(s_lo)
        sm = np.where(valid, seg_global[np.minimum(toks_main, T - 1)] - s_lo,
                      -1.0).astype(np.float32)
        segm[:, 0:128] = sm.reshape(128, 128).T  # segm[p, n] = seg(q=128n+p)
        se = np.full(128, -1.0, np.float32)
        mask_x = toks_extra >= 0
        se[mask_x] = seg_global[toks_extra[mask_x]] - s_lo
        segm[:, 128] = se

        in_maps.append({
            "xSf": xsf, "xSb": xsb,
            "wih_f": wih[0], "wih_b": wih[1],
            "whh_f": whh[0], "whh_b": whh[1],
            "b48": b48, "i48": i48,
            "h0f": h0f.astype(BF16), "c0f": c0f.astype(BF16),
            "h0b": h0b.astype(BF16), "c0b": c0b.astype(BF16),
            "wom": wom, "uo": uo, "iota": iota,
            "identb": identb, "identf": identf,
            "seg": segm,
        })
    return in_maps, s_los


def kernel(**inputs):
    global LAST_RESULT
    from concourse.bass_utils import run_bass_kernel_spmd

    nc = _build()
    in_maps, s_los = _host_prep(inputs)
    res = run_bass_kernel_spmd(nc, in_maps, core_ids=list(range(NCORE)))
    LAST_RESULT = res

    G = np.zeros((S + SWIN, 257), np.float64)
    for c in range(NCORE):
        ctx = np.asarray(res.results[c]["ctx"], np.float32)
        G[s_los[c]:s_los[c] + SWIN] += ctx
    G = G[:S]
    z = G[:, 256]
    ctx = G[:, :256] / np.where(z == 0, 1.0, z)[:, None]
    w_tag = np.asarray(inputs["w_tag"], np.float32)
    b_tag = np.asarray(inputs["b_tag"], np.float32)
    out = ctx.astype(np.float32) @ w_tag.T + b_tag
    return out.astype(np.float32)

